# revision 47
# baseline (speedup 1.0000x reference)
"""Trainium2 Bass kernel for nn_Attention_35734127903400 (v3).

Dense transformer attention block:
  xq = LN(x@wq); xk = LN(x@wk); xv = x@wv          (LN over full flattened head dim)
  rope(q, k); GQA self-attention (16 q heads, 8 kv heads, S=2048, full/non-causal)
  gated cross-attention with y (128 tokens); out = (self + tanh(gate)*cross) @ wo

Sharding (8 cores, no collectives): token-sharded. Core c handles batch
b=c//2, sequence half hf=c%2 (1024 q tokens). Each core computes K/V for
its batch's FULL 2048-token sequence (replicated within the pair), Q only
for its local 1024 tokens. LN is over the feature dim so it is core-local.

v3: the Q/KV/out projections run as fp8e4m3 hi+lo 3-term matmuls in
DoubleRow perf mode (2 contraction k-tiles per instruction at 0.5
cycles/row): x = hi + lo with hi = fp8(x), lo = fp8(x - hi), and
x@w ~= xh@wh + xh@wl + xl@wh. Validated offline at bf16-equivalent
accuracy (the dropped lo*lo term is ~0.1%). Weights pre-scale by WSC=64
into fp8's normal range; the Q/K LN absorbs the scale (eps scaled by
WSC^2), V restores it in its PSUM->SBUF copy. The softmax-path "ones"
matmul uses 1/16 so merged is built x16 for a well-scaled fp8 hi/lo
split feeding the out-proj; the final copy restores /(16*WSC).
Attention scores/AV/exp stay bf16/f32 — fp8 there fails the error
budget (softmax output averages signal down as fast as noise).

Everything stays resident in SBUF — no DRAM spill/reload. Stage order:
y proj (startup filler while wq streams) -> Q proj -> K+V proj (fused,
one x pass) -> attention -> output proj (wo streamed per output chunk).

Scheduling specifics, tuned against the TimelineSim cost model:
- proj chunks: the final contraction round runs the single-buffered
  accumulators first and the PSUM->SBUF copies are emitted in the same
  order, so the next chunk's matmuls never wait on a copy.
- LN affine (Pool) + rope (DVE) run per head-half; head transposes (PE,
  1 cycle/row) trail the rope by 1-3 chunks; the KV stage's last two
  chunks' transposes defer into attention idx 0 (g6/g7) so attention's
  first matmuls never sit behind the rope tail on the in-order PE queue.
- attention: softmax denominator via DVE/Pool pairwise-add tree feeding
  5 ones-matmuls spread through the AV stream; cross-attention matmuls
  (dy/oy) slotted where their PSUM bank is free; exp is the pacing
  engine (ACT); the per-iteration tail (denominator finish + merge)
  defers to g0 of the next iteration so it never delays the next score
  group's exps.
- weight DMAs are per-dc-pair so matmuls start after the first slice;
  wkv prefetches into an untouched right-side SBUF region during Q so
  the KV stage starts without a DMA bubble.
"""

import numpy as np
import ml_dtypes

import concourse.bass as bass
import concourse.mybir as mybir
import concourse.tile as tile
from concourse.bass_utils import run_bass_kernel_spmd
from concourse.masks import make_identity

BF16 = ml_dtypes.bfloat16
F8NP = ml_dtypes.float8_e4m3fn
F32 = mybir.dt.float32
BF = mybir.dt.bfloat16
F8 = mybir.dt.float8e4
WSC = 64.0        # weight pre-scale before fp8 split (w ~ N(0, .02))
DR = mybir.MatmulPerfMode.DoubleRow

P = 128
B, S, D = 4, 2048, 2048
H, KVH = 16, 8
HD = 128
NREP = 2
YL, YD = 128, 1024
EPS = 1e-5
S_LOC = S // 2
DC = D // P          # 16 contraction chunks for D
YDC = YD // P        # 8
TC = S // P          # 16 token chunks (full seq)
TCL = S_LOC // P     # 8 local token chunks
NQ = 512
QCN = S_LOC // NQ    # 2
KVD = KVH * HD       # 1024
SCALE = 1.0 / float(np.sqrt(np.float32(HD)))
AF = mybir.ActivationFunctionType
ALU = mybir.AluOpType

_CACHED = {}
LAST_EXEC_NS = None


def _split_dma_waits(nc, max_waits=1):
    """Hoist excess sync-waits of any instruction onto preceding same-engine
    single-wait NoOps (this build's per-instruction structs have few embedded
    wait slots)."""
    n_split = 0
    for f in nc.m.functions:
        for blk in f.blocks:
            insts = list(blk.instructions)
            out = []
            changed = False
            for ins in insts:
                si = ins.sync_info
                if (si is not None and si.on_wait
                        and len(si.on_wait) > max_waits):
                    waits = list(si.on_wait)
                    for wi, w in enumerate(waits[:-max_waits]):
                        out.append(mybir.InstNoOp(
                            name=f"{ins.name}-wsplit{wi}", engine=ins.engine,
                            sync_info=mybir.SyncInfo(on_wait=[w],
                                                     on_update=[])))
                    ins.sync_info = mybir.SyncInfo(
                        on_wait=waits[-max_waits:],
                        on_update=list(si.on_update))
                    changed = True
                    n_split += 1
                out.append(ins)
            if changed:
                blk.instructions = out
    return n_split


def build_program():
    nc = bass.Bass()

    # ---- I/O (all pre-chunked host-side for >=2KB contiguous runs) ----
    # x and the Q/KV weights ship as fp8 hi+lo pairs (hi = fp8(v),
    # lo = fp8(v - hi)); matmuls run 3-term hi*hi + hi*lo + lo*hi in
    # DoubleRow mode (2 contraction chunks per instruction, 0.5 cyc/row).
    # Weights are pre-scaled by WSC so they sit in fp8's normal range;
    # the Q/K layernorm absorbs the scale, V restores it in the PSUM copy.
    xqh_d = nc.declare_dram_parameter("xqh", [TCL, P, DC, P], F8,
                                      isOutput=False)
    xql_d = nc.declare_dram_parameter("xql", [TCL, P, DC, P], F8,
                                      isOutput=False)
    xh_d = nc.declare_dram_parameter("xh", [TC, P, DC, P], F8, isOutput=False)
    xl_d = nc.declare_dram_parameter("xl", [TC, P, DC, P], F8, isOutput=False)
    y_d = nc.declare_dram_parameter("y", [P, YDC, YL], BF, isOutput=False)
    wqh_d = nc.declare_dram_parameter("wqh", [DC // 2, P, 2, D], F8,
                                      isOutput=False)
    wql_d = nc.declare_dram_parameter("wql", [DC // 2, P, 2, D], F8,
                                      isOutput=False)
    wkvh_d = nc.declare_dram_parameter("wkvh", [DC // 2, P, 2, 2 * KVD], F8,
                                       isOutput=False)
    wkvl_d = nc.declare_dram_parameter("wkvl", [DC // 2, P, 2, 2 * KVD], F8,
                                       isOutput=False)
    wkvy_d = nc.declare_dram_parameter("wkvy", [YDC, P, 2 * KVD], BF,
                                       isOutput=False)
    woh_d = nc.declare_dram_parameter("woh", [DC, P, DC, P], F8,
                                      isOutput=False)
    wol_d = nc.declare_dram_parameter("wol", [DC, P, DC, P], F8,
                                      isOutput=False)
    qw_d = nc.declare_dram_parameter("qw", [D], BF, isOutput=False)
    qb_d = nc.declare_dram_parameter("qb", [D], BF, isOutput=False)
    kw_d = nc.declare_dram_parameter("kw", [KVD], BF, isOutput=False)
    kb_d = nc.declare_dram_parameter("kb", [KVD], BF, isOutput=False)
    kyw_d = nc.declare_dram_parameter("kyw", [KVD], BF, isOutput=False)
    kyb_d = nc.declare_dram_parameter("kyb", [KVD], BF, isOutput=False)
    cosq_d = nc.declare_dram_parameter("cosq", [P, TCL, HD // 2], BF,
                                       isOutput=False)
    sinq_d = nc.declare_dram_parameter("sinq", [P, TCL, HD // 2], BF,
                                       isOutput=False)
    cosk_d = nc.declare_dram_parameter("cosk", [P, TC, HD // 2], BF,
                                       isOutput=False)
    sink_d = nc.declare_dram_parameter("sink", [P, TC, HD // 2], BF,
                                       isOutput=False)
    gates_d = nc.declare_dram_parameter("gates", [H], F32, isOutput=False)
    ymb_d = nc.declare_dram_parameter("ymb", [YL], F32, isOutput=False)
    outT = nc.declare_dram_parameter("outT", [D, S_LOC], F32, isOutput=True)

    with tile.TileContext(nc) as tc:
        from contextlib import ExitStack
        with ExitStack() as ctx:
            cpool = ctx.enter_context(tc.tile_pool(name="consts", bufs=1))
            ident = cpool.tile([P, P], BF)
            make_identity(nc, ident)
            # 1/16 instead of 1.0: scales both softmax reciprocals by 16 so
            # merged is stored x16 (fp8-friendly range); the out-proj copy
            # restores /(16*WSC)
            ones_t = cpool.tile([P, P], BF)
            nc.vector.memset(ones_t, 1.0 / 16.0)
            eps_t = cpool.tile([P, 1], F32)
            nc.vector.memset(eps_t, EPS)
            # Q/K projections come out of the fp8 path scaled by WSC,
            # so their LN stats see var*WSC^2 — scale eps to match.
            eps_s_t = cpool.tile([P, 1], F32)
            nc.vector.memset(eps_s_t, EPS * WSC * WSC)
            gates_t = cpool.tile([P, H], F32)
            nc.gpsimd.dma_start(
                out=gates_t,
                in_=bass.AP(tensor=gates_d, offset=0, ap=[[0, P], [1, H]]))
            ymb_t = cpool.tile([P, 1], F32)
            nc.gpsimd.dma_start(
                out=ymb_t,
                in_=bass.AP(tensor=ymb_d, offset=0, ap=[[1, P], [0, 1]]))

            def bcast_vec(pool, dram_h, n):
                # bf16 vectors halve the stride-0 broadcast DMA bytes that
                # compete with the weight streams at stage starts (numerics:
                # systematic per-feature rounding, measured negligible)
                t = pool.tile([P, n], BF, tag=f"ln_{dram_h.name}", bufs=1)
                nc.gpsimd.dma_start(
                    out=t,
                    in_=bass.AP(tensor=dram_h, offset=0, ap=[[0, P], [1, n]]))
                return t

            def ln_chain(zn, nln, pool, w_t, b_t, pfx, eps=None):
                """stats (DVE) -> rstd/negmr -> normalize (ACT). Affine is
                applied by the caller (per-half on Pool)."""
                stats = pool.tile([P, nln, 6], F32, tag=f"{pfx}bnstats")
                for i in range(nln):
                    nc.vector.bn_stats(out=stats[:, i, :],
                                       in_=zn[:, i * NQ:(i + 1) * NQ])
                mv = pool.tile([P, 2], F32, tag=f"{pfx}bnaggr")
                nc.vector.bn_aggr(out=mv, in_=stats)
                rstd = pool.tile([P, 1], F32, tag=f"{pfx}rstd")
                nc.scalar.activation(out=rstd, in_=mv[:, 1:2],
                                     func=AF.Sqrt,
                                     bias=eps if eps is not None else eps_t,
                                     scale=1.0)
                nc.vector.reciprocal(out=rstd, in_=rstd)
                negmr = pool.tile([P, 1], F32, tag=f"{pfx}negmr")
                nc.vector.tensor_scalar(
                    out=negmr, in0=mv[:, 0:1], scalar1=rstd, scalar2=-1.0,
                    op0=ALU.mult, op1=ALU.mult)
                nc.scalar.activation(out=zn, in_=zn, func=AF.Identity,
                                     scale=rstd, bias=negmr)

            # =========================================================
            # Stage Y: y projections -> YKT (LN, no rope), YV.
            # Runs first: its matmuls fill the PE while wq streams in.
            # =========================================================
            yp = ctx.enter_context(tc.tile_pool(name="ypool", bufs=1))
            YKT = yp.tile([P, KVH, YL], BF)
            YV = yp.tile([P, KVH, HD], BF)
            qtp = ctx.enter_context(tc.tile_pool(name="qtpool", bufs=1))
            QT = qtp.tile([P, H, S_LOC], BF)
            # pre-rope'd y-keys borrow QT head-0 space: the deferred Y
            # transposes (Q hook, chunk 1) read it before Q's first
            # transpose drain (chunk 3) overwrites that region
            ykbf = QT[:, 0, :].rearrange("p (h f) -> p h f", h=KVH)
            lny = tc.alloc_tile_pool(name="lny", bufs=1)
            wY = tc.alloc_tile_pool(name="wY", bufs=1)
            yt = wY.tile([P, YDC, YL], BF, tag="yt")
            nc.sync.dma_start(out=yt, in_=y_d[:, :, :])
            wy_sb = []
            for g in range(YDC):
                wt = wY.tile([P, 2 * KVD], BF, tag=f"wy{g}", name=f"wy{g}")
                nc.sync.dma_start(out=wt, in_=wkvy_d[g])
                wy_sb.append(wt)
            kyw_t = bcast_vec(wY, kyw_d, KVD)
            kyb_t = bcast_vec(wY, kyb_d, KVD)
            psY = tc.alloc_tile_pool(name="psY", bufs=1, space="PSUM")
            ya = [psY.tile([P, NQ], F32, tag=f"ya{n}", bufs=1, name=f"ya{n}")
                  for n in range(4)]
            for dc in range(YDC):
                for n in range(4):
                    nc.tensor.matmul(
                        ya[n][:], lhsT=yt[:, dc, :],
                        rhs=wy_sb[dc][:, n * NQ:(n + 1) * NQ],
                        start=(dc == 0), stop=(dc == YDC - 1))
            ykn = wY.tile([P, KVD], F32, tag="ykn")
            for n in range(2):
                nc.scalar.copy(out=ykn[:, n * NQ:(n + 1) * NQ], in_=ya[n][:])
                nc.scalar.copy(out=YV[:, 4 * n:4 * (n + 1), :],
                               in_=ya[2 + n][:])
            ln_chain(ykn, 2, wY, kyw_t, kyb_t, "y")
            nc.gpsimd.tensor_mul(out=ykn, in0=ykn, in1=kyw_t)
            nc.gpsimd.tensor_add(out=ykn, in0=ykn, in1=kyb_t)
            nc.vector.tensor_copy(out=ykbf, in_=ykn)
            # Y's head transposes are deferred into the Q stage (hook below)
            # so they don't block Q's first matmuls behind the Y LN chain
            psY.release()
            wY.release()
            lny.release()

            # =========================================================
            # shared projection-stage machinery
            # =========================================================
            # final-contraction-round matmul order / copy order: the
            # single-buffered accumulators (2, 3) finish and copy first
            ACC_ORDER = [2, 3, 0, 1]

            def proj_stage(nchunks, x_dram, w_tiles, out_heads,
                           w_t, b_t, cos_dram, sin_dram, dst_T, dst_V, stage,
                           xs, preloaded, hooks, keep_last=0, bfp_ext=None):
                """One pass over `nchunks` token chunks with 4 accumulators.

                fp8 hi/lo path: `x_dram` is an (xh_d, xl_d) pair, `w_tiles` a
                list of DC//2 (w_hi, w_lo) pair-tiles [P, 2, nfeat]. Each
                accumulator gets 3 DoubleRow terms per dc pair (hh, hl, lh);
                all three share one PSUM scale since lo parts are unscaled.

                The first `out_heads*HD` features get LN+rope+transpose into
                dst_T; for the KV stage accumulators [2,3] are V, copied
                (restoring the 1/WSC weight pre-scale) into dst_V[:, t, :].
                `xs`: caller-owned x-tile pool (chunks in `preloaded` were
                DMA'd by the caller before the weight DMAs). `hooks[t]` runs
                after chunk t's x DMA — used to interleave next-stage
                prefetch DMAs into the SP queue.
                """
                nacc = 4
                xh_dram, xl_dram = x_dram
                npair = DC // 2
                bfp = bfp_ext or tc.alloc_tile_pool(name=f"bf{stage}",
                                                    bufs=4)
                csp = tc.alloc_tile_pool(name=f"cs{stage}", bufs=1)
                cs_tiles = {}
                nhalves = (nchunks + 7) // 8

                def load_cs_half(hh):
                    ct = csp.tile([P, 8, HD // 2], BF, tag="ctab",
                                  bufs=nhalves, name=f"ctab{stage}_{hh}")
                    st = csp.tile([P, 8, HD // 2], BF, tag="stab",
                                  bufs=nhalves, name=f"stab{stage}_{hh}")
                    nc.sync.dma_start(out=ct,
                                      in_=cos_dram[:, 8 * hh:8 * hh + 8, :])
                    nc.sync.dma_start(out=st,
                                      in_=sin_dram[:, 8 * hh:8 * hh + 8, :])
                    cs_tiles[hh] = (ct, st)

                load_cs_half(0)
                wk_ = tc.alloc_tile_pool(name=f"work{stage}", bufs=2)
                stp = tc.alloc_tile_pool(name=f"st{stage}", bufs=2)
                rtp = tc.alloc_tile_pool(name=f"rt{stage}", bufs=1)
                psP = tc.alloc_tile_pool(name=f"ps{stage}", bufs=1,
                                         space="PSUM")
                nfeat = out_heads * HD
                nln = nfeat // NQ        # accumulators covered by LN
                oh2 = out_heads // 2     # heads per half
                pending = []             # [(zbf, tok0)]

                def emit_transposes(zbf, tok0):
                    for hg in range(out_heads // 4):
                        h0 = hg * 4
                        tp = psP.tile([P, 4, P], BF, tag="tr", bufs=2)
                        for j in range(4):
                            nc.tensor.transpose(
                                tp[:, j, :], zbf[:, h0 + j, :], ident)
                        nc.scalar.copy(
                            out=dst_T[:, h0:h0 + 4, tok0:tok0 + P],
                            in_=tp)

                def drain(n):
                    for _ in range(n):
                        if pending:
                            emit_transposes(*pending.pop(0))

                for t in range(nchunks):
                    if t in preloaded:
                        xth, xtl = preloaded[t]
                    else:
                        xth = xs.tile([P, DC, P], F8, tag="xth",
                                      name=f"xth{stage}_{t}")
                        nc.sync.dma_start(out=xth, in_=xh_dram[t])
                        xtl = xs.tile([P, DC, P], F8, tag="xtl",
                                      name=f"xtl{stage}_{t}")
                        nc.sync.dma_start(out=xtl, in_=xl_dram[t])
                    if t in hooks:
                        hooks[t](psP)
                    if (t % 8 == 6 and t + 2 < nchunks
                            and (t + 2) // 8 not in cs_tiles):
                        load_cs_half((t + 2) // 8)
                    accs = [psP.tile([P, NQ], F32, tag=f"acc{n}",
                                     bufs=(2 if n < 2 else 1),
                                     name=f"acc{n}_{t}")
                            for n in range(nacc)]
                    # 3 hi/lo terms x npair DoubleRow rounds; last round in
                    # ACC_ORDER so single-buffered accs finish+copy first
                    rounds = []
                    for i in range(npair):
                        rounds.append((xth[:, 2 * i:2 * i + 2, :],
                                       w_tiles[i][0]))
                        rounds.append((xth[:, 2 * i:2 * i + 2, :],
                                       w_tiles[i][1]))
                        rounds.append((xtl[:, 2 * i:2 * i + 2, :],
                                       w_tiles[i][0]))
                    for r, (lt, wt) in enumerate(rounds[:-1]):
                        for n in range(nacc):
                            nc.tensor.matmul(
                                accs[n][:], lhsT=lt,
                                rhs=wt[:, :, n * NQ:(n + 1) * NQ],
                                start=(r == 0), stop=False, perf_mode=DR)
                    lt, wt = rounds[-1]
                    for n in ACC_ORDER:
                        nc.tensor.matmul(
                            accs[n][:], lhsT=lt,
                            rhs=wt[:, :, n * NQ:(n + 1) * NQ],
                            start=False, stop=True, perf_mode=DR)
                    # PSUM -> SBUF copies, staggered order matching the
                    # final round so the next chunk never waits
                    zn = wk_.tile([P, nfeat], F32, tag="work")
                    for n in ACC_ORDER:
                        if n < nln:
                            nc.scalar.copy(out=zn[:, n * NQ:(n + 1) * NQ],
                                           in_=accs[n][:])
                        elif dst_V is not None:
                            nc.scalar.activation(
                                out=dst_V[:, t, (n - nln) * NQ:
                                          (n - nln + 1) * NQ],
                                in_=accs[n][:], func=AF.Copy,
                                scale=1.0 / WSC)
                    # transposes of chunk t-3; their PSUM-drain copies ride
                    # on DVE, which is idle during this chunk's matmuls
                    if t >= 3:
                        drain(1)
                    ln_chain(zn, nln, stp, w_t, b_t, stage, eps=eps_s_t)
                    # affine on Pool + rope on DVE, split into head groups.
                    # The last two chunks split finer: their transposes are
                    # close to the stage tail, so a shorter serial chain
                    # (affine part -> rope part pipelined across engines)
                    # directly shortens the stage-exit stall.
                    nsplit = 4 if t >= nchunks - 2 else 2
                    ohs = out_heads // nsplit
                    zbf = bfp.tile([P, out_heads, HD], BF, tag="zbf")
                    zv = zn.rearrange("p (h f two) -> p h f two",
                                      h=out_heads, two=2)
                    zb = zbf.rearrange("p h (f two) -> p h f two", two=2)
                    ct_t, st_t = cs_tiles[t // 8]
                    shp = (P, ohs, HD // 2)
                    cb = ct_t[:, t % 8, :][:, None, :].to_broadcast(shp)
                    sb = st_t[:, t % 8, :][:, None, :].to_broadcast(shp)
                    for part in range(nsplit):
                        f0 = part * (nfeat // nsplit)
                        f1 = (part + 1) * (nfeat // nsplit)
                        nc.gpsimd.tensor_mul(out=zn[:, f0:f1],
                                             in0=zn[:, f0:f1],
                                             in1=w_t[:, f0:f1])
                        nc.gpsimd.tensor_add(out=zn[:, f0:f1],
                                             in0=zn[:, f0:f1],
                                             in1=b_t[:, f0:f1])
                        h0, h1 = part * ohs, (part + 1) * ohs
                        re = zv[:, h0:h1, :, 0]
                        im = zv[:, h0:h1, :, 1]
                        rebf = zb[:, h0:h1, :, 0]
                        imbf = zb[:, h0:h1, :, 1]
                        t1 = rtp.tile([P, ohs, HD // 2], F32, tag="r1")
                        t2 = rtp.tile([P, ohs, HD // 2], F32, tag="r2")
                        nc.vector.tensor_mul(out=t1, in0=re, in1=cb)
                        nc.vector.tensor_mul(out=t2, in0=im, in1=sb)
                        nc.vector.tensor_sub(out=rebf, in0=t1, in1=t2)
                        nc.vector.tensor_mul(out=t1, in0=re, in1=sb)
                        nc.vector.tensor_mul(out=t2, in0=im, in1=cb)
                        nc.vector.tensor_add(out=imbf, in0=t1, in1=t2)
                    pending.append((zbf, t * P))
                while len(pending) > keep_last:
                    drain(1)
                for pool in (psP, rtp, stp, wk_, csp):
                    pool.release()
                if keep_last == 0:
                    if bfp_ext is None:
                        bfp.release()
                    return [], None
                # bfp (bottom of this stage's transient stack) stays alive
                # so the caller can emit the kept chunks' transposes later
                return pending, bfp

            # =========================================================
            # Stage Q: local-half Q projection
            # =========================================================
            lnq = tc.alloc_tile_pool(name="lnq", bufs=1)
            qw_t = bcast_vec(lnq, qw_d, D)
            qb_t = bcast_vec(lnq, qb_d, D)
            xsQ = tc.alloc_tile_pool(name="xsQ", bufs=2)
            xq0h = xsQ.tile([P, DC, P], F8, tag="xth", name="xthQ_0")
            nc.sync.dma_start(out=xq0h, in_=xqh_d[0])
            xq0l = xsQ.tile([P, DC, P], F8, tag="xtl", name="xtlQ_0")
            nc.sync.dma_start(out=xq0l, in_=xql_d[0])
            xq1h = xsQ.tile([P, DC, P], F8, tag="xth", name="xthQ_1")
            nc.sync.dma_start(out=xq1h, in_=xqh_d[1])
            xq1l = xsQ.tile([P, DC, P], F8, tag="xtl", name="xtlQ_1")
            nc.sync.dma_start(out=xq1l, in_=xql_d[1])
            wQ = tc.alloc_tile_pool(name="wQ", bufs=1)
            wq_sb = []
            for g in range(DC // 2):
                wth = wQ.tile([P, 2, D], F8, tag=f"wqh{g}", name=f"wqh{g}")
                nc.sync.dma_start(out=wth, in_=wqh_d[g])
                wtl = wQ.tile([P, 2, D], F8, tag=f"wql{g}", name=f"wql{g}")
                nc.sync.dma_start(out=wtl, in_=wql_d[g])
                wq_sb.append((wth, wtl))

            # prefetch first half of wkv into untouched right-side SBUF
            wKVa = tc.alloc_tile_pool(name="wKVa", bufs=1, side="right")
            wkv_sb = [None] * (DC // 2)

            def hook_ytr(psP):
                for hg in range(2):
                    tp = psP.tile([P, 4, P], BF, tag="tr", bufs=2,
                                  name=f"ytr{hg}")
                    for j in range(4):
                        nc.tensor.transpose(
                            tp[:, j, :], ykbf[:, hg * 4 + j, :], ident)
                    nc.scalar.copy(
                        out=YKT[:, hg * 4:(hg + 1) * 4, :], in_=tp)

            def hook_wkva(psP):
                for g in range(4):
                    wth = wKVa.tile([P, 2, 2 * KVD], F8, tag=f"wkvh{g}",
                                    name=f"wkvh{g}")
                    nc.sync.dma_start(out=wth, in_=wkvh_d[g])
                    wtl = wKVa.tile([P, 2, 2 * KVD], F8, tag=f"wkvl{g}",
                                    name=f"wkvl{g}")
                    nc.sync.dma_start(out=wtl, in_=wkvl_d[g])
                    wkv_sb[g] = (wth, wtl)

            proj_stage(TCL, (xqh_d, xql_d), wq_sb, H, qw_t, qb_t,
                       cosq_d, sinq_d, QT, None, "Q",
                       xsQ, {0: (xq0h, xq0l), 1: (xq1h, xq1l)},
                       {1: hook_ytr, 3: hook_wkva})
            wQ.release()
            xsQ.release()
            lnq.release()

            # =========================================================
            # Stage KV: full-seq K (LN+rope) and V projections, one x pass
            # =========================================================
            ktvp = ctx.enter_context(tc.tile_pool(name="ktvpool", bufs=1))
            KT = ktvp.tile([P, KVH, S], BF)
            Vsb = ktvp.tile([P, TC, KVD], BF)
            lnk = tc.alloc_tile_pool(name="lnk", bufs=1)
            kw_t = bcast_vec(lnk, kw_d, KVD)
            kb_t = bcast_vec(lnk, kb_d, KVD)
            # x tiles ahead of the wkv-second-half DMAs in the SP queue
            xsK = tc.alloc_tile_pool(name="xsK", bufs=2)
            xk0h = xsK.tile([P, DC, P], F8, tag="xth", name="xthK_0")
            nc.sync.dma_start(out=xk0h, in_=xh_d[0])
            xk0l = xsK.tile([P, DC, P], F8, tag="xtl", name="xtlK_0")
            nc.sync.dma_start(out=xk0l, in_=xl_d[0])
            xk1h = xsK.tile([P, DC, P], F8, tag="xth", name="xthK_1")
            nc.sync.dma_start(out=xk1h, in_=xh_d[1])
            xk1l = xsK.tile([P, DC, P], F8, tag="xtl", name="xtlK_1")
            nc.sync.dma_start(out=xk1l, in_=xl_d[1])
            # second wkv half into fresh right-side space: its DMAs have no
            # space-dependency on the Q stage and start immediately
            wKVb = tc.alloc_tile_pool(name="wKVb", bufs=1, side="right")
            for g in range(4, DC // 2):
                wth = wKVb.tile([P, 2, 2 * KVD], F8, tag=f"wkvh{g}",
                                name=f"wkvh{g}")
                nc.sync.dma_start(out=wth, in_=wkvh_d[g])
                wtl = wKVb.tile([P, 2, 2 * KVD], F8, tag=f"wkvl{g}",
                                name=f"wkvl{g}")
                nc.sync.dma_start(out=wtl, in_=wkvl_d[g])
                wkv_sb[g] = (wth, wtl)
            kv_pending, bfK = proj_stage(TC, (xh_d, xl_d), wkv_sb, KVH,
                                         kw_t, kb_t,
                                         cosk_d, sink_d, KT, Vsb, "K",
                                         xsK, {0: (xk0h, xk0l),
                                               1: (xk1h, xk1l)}, keep_last=2,
                                         hooks={})
            wKVb.release()
            wKVa.release()

            # =========================================================
            # Stage attention: per (head, q-chunk)
            # =========================================================
            mgp = tc.alloc_tile_pool(name="merged", bufs=1)
            merged_h = mgp.tile([P, H, S_LOC], F8)
            merged_l = mgp.tile([P, H, S_LOC], F8)
            wop = tc.alloc_tile_pool(name="wop", bufs=3)
            ep = tc.alloc_tile_pool(name="epool", bufs=5)
            esp = tc.alloc_tile_pool(name="espool", bufs=2)
            eyp = tc.alloc_tile_pool(name="eypool", bufs=2)
            rcp = tc.alloc_tile_pool(name="rcpool", bufs=1)
            psA = tc.alloc_tile_pool(name="psA", bufs=1, space="PSUM")
            outp = tc.alloc_tile_pool(name="outp", bufs=2)
            # prefetch first wo slices during attention
            wo_tiles = {}
            for oc in range(2):
                wo_th = wop.tile([P, DC, P], F8, tag="woh", name=f"woh{oc}")
                nc.sync.dma_start(out=wo_th, in_=woh_d[oc])
                wo_tl = wop.tile([P, DC, P], F8, tag="wol", name=f"wol{oc}")
                nc.sync.dma_start(out=wo_tl, in_=wol_d[oc])
                wo_tiles[oc] = (wo_th, wo_tl)

            def emit_outproj_block(oc, qc):
                """24 DoubleRow matmuls into an o-ring PSUM slot + DVE
                scaled copy + DMA out. ACT-free so it absorbs the exp
                backlog when interleaved."""
                if oc in wo_tiles:
                    wo_th, wo_tl = wo_tiles.pop(oc)
                else:
                    wo_th = wop.tile([P, DC, P], F8, tag="woh",
                                     name=f"woh{oc}_{qc}")
                    nc.sync.dma_start(out=wo_th, in_=woh_d[oc])
                    wo_tl = wop.tile([P, DC, P], F8, tag="wol",
                                     name=f"wol{oc}_{qc}")
                    nc.sync.dma_start(out=wo_tl, in_=wol_d[oc])
                q0 = qc * NQ
                out_ps = psA.tile([P, NQ], F32, tag="o", bufs=2,
                                  name=f"ops{oc}_{qc}")
                rounds = []
                for i in range(DC // 2):
                    sl = slice(2 * i, 2 * i + 2)
                    rounds.append((wo_th[:, sl, :], merged_h[:, sl,
                                                            q0:q0 + NQ]))
                    rounds.append((wo_tl[:, sl, :], merged_h[:, sl,
                                                            q0:q0 + NQ]))
                    rounds.append((wo_th[:, sl, :], merged_l[:, sl,
                                                            q0:q0 + NQ]))
                for r, (lt, rt) in enumerate(rounds):
                    nc.tensor.matmul(
                        out_ps[:], lhsT=lt, rhs=rt,
                        start=(r == 0), stop=(r == len(rounds) - 1),
                        perf_mode=DR)
                out_t = outp.tile([P, NQ], F32, tag="outt")
                nc.vector.tensor_scalar_mul(out_t, out_ps[:],
                                            1.0 / (16.0 * WSC))
                nc.sync.dma_start(
                    out=outT[oc * P:(oc + 1) * P, q0:q0 + NQ],
                    in_=out_t)

            # cross scores for iteration idx+1 are issued at g5 of idx so
            # ACT's Ey exp never waits on a cold sy matmul
            def emit_sy(idx):
                qc, h = divmod(idx, H)
                kv = h // NREP
                sy = psA.tile([P, NQ], F32, tag="cr", bufs=1,
                              name=f"sy{idx}")
                nc.tensor.matmul(sy[:], lhsT=YKT[:, kv, :],
                                 rhs=QT[:, h, qc * NQ:qc * NQ + NQ],
                                 start=True, stop=True,
                                 skip_group_check=True)
                Ey = eyp.tile([P, NQ], BF, tag="Ey", name=f"Ey{idx}")
                nc.scalar.activation(out=Ey, in_=sy[:], func=AF.Exp,
                                     scale=SCALE, bias=ymb_t)
                return Ey

            def emit_tail(st):
                """Denominator tail + merge of the previous iteration,
                emitted after the next iteration's first score group so the
                exp backlog never blocks the PE at iteration boundaries."""
                es8_, d_, o_, t1_, h_, q0_ = st
                nc.tensor.matmul(d_[:], lhsT=ones_t, rhs=es8_[:, 2, :],
                                 start=False, stop=False,
                                 skip_group_check=True)
                nc.tensor.matmul(d_[:], lhsT=ones_t, rhs=es8_[:, 6, :],
                                 start=False, stop=False,
                                 skip_group_check=True)
                nc.tensor.matmul(d_[:], lhsT=ones_t, rhs=es8_[:, 7, :],
                                 start=False, stop=True,
                                 skip_group_check=True)
                rec = rcp.tile([P, NQ], F32, tag="rec")
                nc.vector.reciprocal(out=rec, in_=d_[:])
                t0 = rcp.tile([P, NQ], F32, tag="t0")
                nc.vector.tensor_mul(out=t0, in0=o_[:], in1=rec)
                # merged (x16 via the 1/16 ones) as fp8 hi + lo residual;
                # t0 doubles as the f32 sum (in-place add)
                nc.gpsimd.tensor_add(out=t0, in0=t0, in1=t1_)
                nc.gpsimd.tensor_copy(out=merged_h[:, h_, q0_:q0_ + NQ],
                                      in_=t0)
                nc.gpsimd.tensor_sub(out=merged_l[:, h_, q0_:q0_ + NQ],
                                     in0=t0,
                                     in1=merged_h[:, h_, q0_:q0_ + NQ])

            NIT = QCN * H
            Ey = emit_sy(0)
            deferred = None
            for idx in range(NIT):
                qc, h = divmod(idx, H)
                kv = h // NREP
                q0 = qc * NQ
                qt = QT[:, h, q0:q0 + NQ]
                o_ps = psA.tile([P, NQ], F32, tag="o", bufs=2)
                d_ps = None
                es8 = esp.tile([P, 8, NQ], BF, tag="es8")
                for g in range(8):
                    if idx == 0 and g >= 6 and kv_pending:
                        # chunks 14/15's K transposes, deferred past their
                        # rope chains; first consumer is g7's score mms
                        zbf15, ktok = kv_pending.pop(0)
                        for hg in range(2):
                            tp = psA.tile([P, 4, P], BF, tag="s", bufs=2,
                                          name=f"ktr{ktok}_{hg}")
                            for j in range(4):
                                nc.tensor.transpose(
                                    tp[:, j, :], zbf15[:, hg * 4 + j, :],
                                    ident)
                            nc.scalar.copy(
                                out=KT[:, hg * 4:(hg + 1) * 4,
                                       ktok:ktok + P],
                                in_=tp)
                    s_ps = psA.tile([P, 2, NQ], F32, tag="s", bufs=2)
                    for j in range(2):
                        kc = 2 * g + j
                        nc.tensor.matmul(
                            s_ps[:, j, :],
                            lhsT=KT[:, kv, kc * P:(kc + 1) * P],
                            rhs=qt, start=True, stop=True,
                            skip_group_check=True)
                    E = ep.tile([P, 2, NQ], BF, tag="E")
                    nc.scalar.activation(out=E, in_=s_ps[:],
                                         func=AF.Exp, scale=SCALE)
                    nc.vector.tensor_add(out=es8[:, g, :],
                                         in0=E[:, 0, :], in1=E[:, 1, :])
                    if g == 0 and deferred is not None:
                        emit_tail(deferred)
                        deferred = None
                    if g in (1, 3, 5):
                        # lvl1 tree add on Pool (in-place into es8[0:3])
                        i = g // 2
                        nc.gpsimd.tensor_add(out=es8[:, i, :],
                                             in0=es8[:, 2 * i, :],
                                             in1=es8[:, 2 * i + 1, :])
                    for j in range(2):
                        kc = 2 * g + j
                        nc.tensor.matmul(
                            o_ps[:],
                            lhsT=Vsb[:, kc, kv * HD:(kv + 1) * HD],
                            rhs=E[:, j, :],
                            start=(kc == 0), stop=(kc == TC - 1),
                            skip_group_check=True)
                    if g == 1:
                        dy = psA.tile([P, NQ], F32, tag="cr", bufs=1,
                                      name=f"dy{idx}")
                        nc.tensor.matmul(dy[:], lhsT=ones_t, rhs=Ey,
                                         start=True, stop=True,
                                         skip_group_check=True)
                        rec_y = rcp.tile([P, NQ], F32, tag="recy")
                        nc.vector.reciprocal(out=rec_y, in_=dy[:])
                    elif g == 3:
                        d_ps = psA.tile([P, NQ], F32, tag="d", bufs=1,
                                        name=f"d{idx}")
                        nc.tensor.matmul(
                            d_ps[:], lhsT=ones_t, rhs=es8[:, 0, :],
                            start=True, stop=False,
                            skip_group_check=True)
                        oy = psA.tile([P, NQ], F32, tag="cr", bufs=1,
                                      name=f"oy{idx}")
                        nc.tensor.matmul(oy[:], lhsT=YV[:, kv, :],
                                         rhs=Ey, start=True, stop=True,
                                         skip_group_check=True)
                        t1 = rcp.tile([P, NQ], F32, tag="t1", bufs=2)
                        nc.vector.scalar_tensor_tensor(
                            out=t1, in0=oy[:],
                            scalar=gates_t[:, h:h + 1],
                            in1=rec_y, op0=ALU.mult, op1=ALU.mult)
                    elif g == 5:
                        nc.tensor.matmul(
                            d_ps[:], lhsT=ones_t, rhs=es8[:, 1, :],
                            start=False, stop=False,
                            skip_group_check=True)
                        if idx + 1 < NIT:
                            next_Ey = emit_sy(idx + 1)

                deferred = (es8, d_ps, o_ps, t1, h, q0)
                Ey = next_Ey
                # during the second q-chunk pass, interleave the first
                # q-chunk's output projection (ACT-free PE work)
                if qc == 1:
                    emit_outproj_block(h, 0)

            if deferred is not None:
                emit_tail(deferred)
                deferred = None
            # remaining out-proj: second q-chunk
            for oc in range(DC):
                emit_outproj_block(oc, 1)
            psA.release()
            outp.release()
            rcp.release()
            eyp.release()
            esp.release()
            ep.release()
            wop.release()
            mgp.release()
            bfK.release()
            xsK.release()
            lnk.release()

    _split_dma_waits(nc)
    return nc


def _prep_inputs(x, y, freqs_cos, freqs_sin, y_mask, wq, wk, wv, wk_y, wv_y,
                 wo, q_w, q_b, k_w, k_b, ky_w, ky_b, gate):
    f32 = np.float32

    def hilo(v):
        hi = v.astype(F8NP)
        lo = (v - hi.astype(f32)).astype(F8NP)
        return hi, lo

    def chunk_x(xb):
        # [S, D] -> [tc, p, dc, s]: out[t, p, dc, s] = xb[t*128+s, dc*128+p]
        # as an fp8 (hi, lo) pair
        t = xb.shape[0] // P
        ch = np.ascontiguousarray(
            xb.reshape(t, P, DC, P).transpose(0, 3, 2, 1))
        return hilo(ch)

    def pair_w(wm):
        # [D, F] -> fp8 hi/lo pairs [DC/2, p, 2, F] (WSC pre-scale)
        fdim = wm.shape[1]
        ws = np.ascontiguousarray(
            (np.asarray(wm, f32) * WSC).reshape(DC // 2, 2, P, fdim)
            .transpose(0, 2, 1, 3))
        return hilo(ws)

    def chunk_cs(tab):
        # [S', 64] -> [p, t, f]
        t = tab.shape[0] // P
        return np.ascontiguousarray(
            np.asarray(tab, f32).reshape(t, P, HD // 2).transpose(1, 0, 2)
            .astype(BF16))

    wo_s = np.ascontiguousarray(
        (np.asarray(wo, f32) * WSC).reshape(DC, P, DC, P)
        .transpose(2, 1, 0, 3))
    woh8 = wo_s.astype(F8NP)
    wol8 = (wo_s - woh8.astype(f32)).astype(F8NP)
    wqh, wql = pair_w(np.asarray(wq, f32))
    wkvh, wkvl = pair_w(np.concatenate(
        [np.asarray(wk, f32), np.asarray(wv, f32)], axis=1))
    shared = {
        "wqh": wqh,
        "wql": wql,
        "wkvh": wkvh,
        "wkvl": wkvl,
        "wkvy": np.ascontiguousarray(np.concatenate(
            [np.asarray(wk_y, f32), np.asarray(wv_y, f32)],
            axis=1).astype(BF16).reshape(YDC, P, 2 * KVD)),
        "woh": woh8,
        "wol": wol8,
        "qw": np.ascontiguousarray(np.asarray(q_w, f32).astype(BF16)),
        "qb": np.ascontiguousarray(np.asarray(q_b, f32).astype(BF16)),
        "kw": np.ascontiguousarray(np.asarray(k_w, f32).astype(BF16)),
        "kb": np.ascontiguousarray(np.asarray(k_b, f32).astype(BF16)),
        "kyw": np.ascontiguousarray(np.asarray(ky_w, f32).astype(BF16)),
        "kyb": np.ascontiguousarray(np.asarray(ky_b, f32).astype(BF16)),
        "cosk": chunk_cs(freqs_cos),
        "sink": chunk_cs(freqs_sin),
        "gates": np.ascontiguousarray(np.tanh(np.asarray(gate, f32))),
    }
    per_core = []
    for c in range(8):
        b, hf = c // 2, c % 2
        sl = slice(hf * S_LOC, (hf + 1) * S_LOC)
        xb = np.asarray(x[b], f32)
        m = dict(shared)
        m["xh"], m["xl"] = chunk_x(xb)
        m["xqh"], m["xql"] = chunk_x(xb[sl])
        m["y"] = np.ascontiguousarray(
            np.asarray(y[b], f32).T.astype(BF16).reshape(YDC, P, YL)
            .transpose(1, 0, 2))
        m["cosq"] = chunk_cs(np.asarray(freqs_cos, f32)[sl])
        m["sinq"] = chunk_cs(np.asarray(freqs_sin, f32)[sl])
        m["ymb"] = np.where(np.asarray(y_mask[b]), 0.0, -1e9).astype(f32)
        per_core.append(m)
    return per_core


def kernel(**inputs):
    if "nc" not in _CACHED:
        _CACHED["nc"] = build_program()
    nc = _CACHED["nc"]
    in_maps = _prep_inputs(
        inputs["x"], inputs["y"], inputs["freqs_cos"], inputs["freqs_sin"],
        inputs["y_mask"], inputs["wq"], inputs["wk"], inputs["wv"],
        inputs["wk_y"], inputs["wv_y"], inputs["wo"], inputs["q_w"],
        inputs["q_b"], inputs["k_w"], inputs["k_b"], inputs["ky_w"],
        inputs["ky_b"], inputs["gate"])
    res = run_bass_kernel_spmd(nc, in_maps, core_ids=list(range(8)))
    global LAST_EXEC_NS
    LAST_EXEC_NS = res.exec_time_ns
    out = np.zeros((B, S, D), np.float32)
    for c in range(8):
        b, hf = c // 2, c % 2
        out[b, hf * S_LOC:(hf + 1) * S_LOC, :] = res.results[c]["outT"].T
    return out


if __name__ == "__main__":
    nc = build_program()
    print("program built OK")



# revision 56
# speedup vs baseline: 1.0205x; 1.0205x over previous
"""Trainium2 Bass kernel for nn_Attention_35734127903400 (v3).

Dense transformer attention block:
  xq = LN(x@wq); xk = LN(x@wk); xv = x@wv          (LN over full flattened head dim)
  rope(q, k); GQA self-attention (16 q heads, 8 kv heads, S=2048, full/non-causal)
  gated cross-attention with y (128 tokens); out = (self + tanh(gate)*cross) @ wo

Sharding (8 cores, no collectives): token-sharded. Core c handles batch
b=c//2, sequence half hf=c%2 (1024 q tokens). Each core computes K/V for
its batch's FULL 2048-token sequence (replicated within the pair), Q only
for its local 1024 tokens. LN is over the feature dim so it is core-local.

v3: the Q/KV/out projections run as fp8e4m3 hi+lo 3-term matmuls in
DoubleRow perf mode (2 contraction k-tiles per instruction at 0.5
cycles/row): x = hi + lo with hi = fp8(x), lo = fp8(x - hi), and
x@w ~= xh@wh + xh@wl + xl@wh. Validated offline at bf16-equivalent
accuracy (the dropped lo*lo term is ~0.1%). Weights pre-scale by WSC=64
into fp8's normal range; the Q/K LN absorbs the scale (eps scaled by
WSC^2), V restores it in its PSUM->SBUF copy. The softmax-path "ones"
matmul uses 1/16 so merged is built x16 for a well-scaled fp8 hi/lo
split feeding the out-proj; the final copy restores /(16*WSC).
Attention scores/AV/exp stay bf16/f32 — fp8 there fails the error
budget (softmax output averages signal down as fast as noise).

Everything stays resident in SBUF — no DRAM spill/reload. Stage order:
y proj (startup filler while wq streams) -> Q proj -> K+V proj (fused,
one x pass) -> attention -> output proj (wo streamed per output chunk).

Scheduling specifics, tuned against the TimelineSim cost model:
- proj chunks: the final contraction round runs the single-buffered
  accumulators first and the PSUM->SBUF copies are emitted in the same
  order, so the next chunk's matmuls never wait on a copy.
- LN affine (Pool) + rope (DVE) run per head-half; head transposes (PE,
  1 cycle/row) trail the rope by 1-3 chunks; the KV stage's last two
  chunks' transposes defer into attention idx 0 (g6/g7) so attention's
  first matmuls never sit behind the rope tail on the in-order PE queue.
- attention: softmax denominator via DVE/Pool pairwise-add tree feeding
  5 ones-matmuls spread through the AV stream; cross-attention matmuls
  (dy/oy) slotted where their PSUM bank is free; exp is the pacing
  engine (ACT); the per-iteration tail (denominator finish + merge)
  defers to g0 of the next iteration so it never delays the next score
  group's exps.
- weight DMAs are per-dc-pair so matmuls start after the first slice;
  wkv prefetches into an untouched right-side SBUF region during Q so
  the KV stage starts without a DMA bubble.
"""

import numpy as np
import ml_dtypes

import concourse.bass as bass
import concourse.mybir as mybir
import concourse.tile as tile
from concourse.bass_utils import run_bass_kernel_spmd
from concourse.masks import make_identity

BF16 = ml_dtypes.bfloat16
F8NP = ml_dtypes.float8_e4m3fn
F32 = mybir.dt.float32
BF = mybir.dt.bfloat16
F8 = mybir.dt.float8e4
WSC = 64.0        # weight pre-scale before fp8 split (w ~ N(0, .02))
DR = mybir.MatmulPerfMode.DoubleRow

P = 128
B, S, D = 4, 2048, 2048
H, KVH = 16, 8
HD = 128
NREP = 2
YL, YD = 128, 1024
EPS = 1e-5
S_LOC = S // 2
DC = D // P          # 16 contraction chunks for D
YDC = YD // P        # 8
TC = S // P          # 16 token chunks (full seq)
TCL = S_LOC // P     # 8 local token chunks
NQ = 512
QCN = S_LOC // NQ    # 2
KVD = KVH * HD       # 1024
SCALE = 1.0 / float(np.sqrt(np.float32(HD)))
AF = mybir.ActivationFunctionType
ALU = mybir.AluOpType

_CACHED = {}
LAST_EXEC_NS = None


def _split_dma_waits(nc, max_waits=1):
    """Hoist excess sync-waits of any instruction onto preceding same-engine
    single-wait NoOps (this build's per-instruction structs have few embedded
    wait slots)."""
    n_split = 0
    for f in nc.m.functions:
        for blk in f.blocks:
            insts = list(blk.instructions)
            out = []
            changed = False
            for ins in insts:
                si = ins.sync_info
                if (si is not None and si.on_wait
                        and len(si.on_wait) > max_waits):
                    waits = list(si.on_wait)
                    for wi, w in enumerate(waits[:-max_waits]):
                        out.append(mybir.InstNoOp(
                            name=f"{ins.name}-wsplit{wi}", engine=ins.engine,
                            sync_info=mybir.SyncInfo(on_wait=[w],
                                                     on_update=[])))
                    ins.sync_info = mybir.SyncInfo(
                        on_wait=waits[-max_waits:],
                        on_update=list(si.on_update))
                    changed = True
                    n_split += 1
                out.append(ins)
            if changed:
                blk.instructions = out
    return n_split


def build_program():
    nc = bass.Bass()

    # ---- I/O (all pre-chunked host-side for >=2KB contiguous runs) ----
    # x and the Q/KV weights ship as fp8 hi+lo pairs (hi = fp8(v),
    # lo = fp8(v - hi)); matmuls run 3-term hi*hi + hi*lo + lo*hi in
    # DoubleRow mode (2 contraction chunks per instruction, 0.5 cyc/row).
    # Weights are pre-scaled by WSC so they sit in fp8's normal range;
    # the Q/K layernorm absorbs the scale, V restores it in the PSUM copy.
    xqh_d = nc.declare_dram_parameter("xqh", [TCL, P, DC, P], F8,
                                      isOutput=False)
    xql_d = nc.declare_dram_parameter("xql", [TCL, P, DC, P], F8,
                                      isOutput=False)
    xh_d = nc.declare_dram_parameter("xh", [TC, P, DC, P], F8, isOutput=False)
    xl_d = nc.declare_dram_parameter("xl", [TC, P, DC, P], F8, isOutput=False)
    y_d = nc.declare_dram_parameter("y", [P, YDC, YL], BF, isOutput=False)
    wqh_d = nc.declare_dram_parameter("wqh", [DC // 2, P, 2, D], F8,
                                      isOutput=False)
    wql_d = nc.declare_dram_parameter("wql", [DC // 2, P, 2, D], F8,
                                      isOutput=False)
    wkvh_d = nc.declare_dram_parameter("wkvh", [DC // 2, P, 2, 2 * KVD], F8,
                                       isOutput=False)
    wkvl_d = nc.declare_dram_parameter("wkvl", [DC // 2, P, 2, 2 * KVD], F8,
                                       isOutput=False)
    wkvy_d = nc.declare_dram_parameter("wkvy", [YDC, P, 2 * KVD], BF,
                                       isOutput=False)
    woh_d = nc.declare_dram_parameter("woh", [DC, P, DC, P], F8,
                                      isOutput=False)
    wol_d = nc.declare_dram_parameter("wol", [DC, P, DC, P], F8,
                                      isOutput=False)
    qw_d = nc.declare_dram_parameter("qw", [D], BF, isOutput=False)
    qb_d = nc.declare_dram_parameter("qb", [D], BF, isOutput=False)
    kw_d = nc.declare_dram_parameter("kw", [KVD], BF, isOutput=False)
    kb_d = nc.declare_dram_parameter("kb", [KVD], BF, isOutput=False)
    kyw_d = nc.declare_dram_parameter("kyw", [KVD], BF, isOutput=False)
    kyb_d = nc.declare_dram_parameter("kyb", [KVD], BF, isOutput=False)
    cosq_d = nc.declare_dram_parameter("cosq", [P, TCL, HD // 2], BF,
                                       isOutput=False)
    sinq_d = nc.declare_dram_parameter("sinq", [P, TCL, HD // 2], BF,
                                       isOutput=False)
    cosk_d = nc.declare_dram_parameter("cosk", [P, TC, HD // 2], BF,
                                       isOutput=False)
    sink_d = nc.declare_dram_parameter("sink", [P, TC, HD // 2], BF,
                                       isOutput=False)
    gates_d = nc.declare_dram_parameter("gates", [H], F32, isOutput=False)
    ymb_d = nc.declare_dram_parameter("ymb", [YL], F32, isOutput=False)
    outT = nc.declare_dram_parameter("outT", [D, S_LOC], F32, isOutput=True)

    with tile.TileContext(nc) as tc:
        from contextlib import ExitStack
        with ExitStack() as ctx:
            cpool = ctx.enter_context(tc.tile_pool(name="consts", bufs=1))
            ident = cpool.tile([P, P], BF)
            make_identity(nc, ident)
            # 1/16 instead of 1.0: scales both softmax reciprocals by 16 so
            # merged is stored x16 (fp8-friendly range); the out-proj copy
            # restores /(16*WSC)
            ones_t = cpool.tile([P, P], BF)
            nc.vector.memset(ones_t, 1.0 / 16.0)
            eps_t = cpool.tile([P, 1], F32)
            nc.vector.memset(eps_t, EPS)
            # Q/K projections come out of the fp8 path scaled by WSC,
            # so their LN stats see var*WSC^2 — scale eps to match.
            eps_s_t = cpool.tile([P, 1], F32)
            nc.vector.memset(eps_s_t, EPS * WSC * WSC)
            gates_t = cpool.tile([P, H], F32)
            nc.gpsimd.dma_start(
                out=gates_t,
                in_=bass.AP(tensor=gates_d, offset=0, ap=[[0, P], [1, H]]))
            ymb_t = cpool.tile([P, 1], F32)
            nc.gpsimd.dma_start(
                out=ymb_t,
                in_=bass.AP(tensor=ymb_d, offset=0, ap=[[1, P], [0, 1]]))

            def bcast_vec(pool, dram_h, n):
                # bf16 vectors halve the stride-0 broadcast DMA bytes that
                # compete with the weight streams at stage starts (numerics:
                # systematic per-feature rounding, measured negligible)
                t = pool.tile([P, n], BF, tag=f"ln_{dram_h.name}", bufs=1)
                nc.gpsimd.dma_start(
                    out=t,
                    in_=bass.AP(tensor=dram_h, offset=0, ap=[[0, P], [1, n]]))
                return t

            def ln_chain(zn, nln, pool, w_t, b_t, pfx, eps=None):
                """stats (DVE) -> rstd/negmr -> normalize (ACT). Affine is
                applied by the caller (per-half on Pool)."""
                stats = pool.tile([P, nln, 6], F32, tag=f"{pfx}bnstats")
                for i in range(nln):
                    nc.vector.bn_stats(out=stats[:, i, :],
                                       in_=zn[:, i * NQ:(i + 1) * NQ])
                mv = pool.tile([P, 2], F32, tag=f"{pfx}bnaggr")
                nc.vector.bn_aggr(out=mv, in_=stats)
                rstd = pool.tile([P, 1], F32, tag=f"{pfx}rstd")
                nc.scalar.activation(out=rstd, in_=mv[:, 1:2],
                                     func=AF.Sqrt,
                                     bias=eps if eps is not None else eps_t,
                                     scale=1.0)
                nc.vector.reciprocal(out=rstd, in_=rstd)
                negmr = pool.tile([P, 1], F32, tag=f"{pfx}negmr")
                nc.vector.tensor_scalar(
                    out=negmr, in0=mv[:, 0:1], scalar1=rstd, scalar2=-1.0,
                    op0=ALU.mult, op1=ALU.mult)
                nc.scalar.activation(out=zn, in_=zn, func=AF.Identity,
                                     scale=rstd, bias=negmr)

            # =========================================================
            # Stage Y: y projections -> YKT (LN, no rope), YV.
            # Runs first: its matmuls fill the PE while wq streams in.
            # =========================================================
            yp = ctx.enter_context(tc.tile_pool(name="ypool", bufs=1))
            YKT = yp.tile([P, KVH, YL], BF)
            YV = yp.tile([P, KVH, HD], BF)
            qtp = ctx.enter_context(tc.tile_pool(name="qtpool", bufs=1))
            QT = qtp.tile([P, H, S_LOC], BF)
            # Q-stage rope-output ring lives low in the stack so it can
            # outlive wQ/xsQ (its last 2 chunks' transposes defer into the
            # KV stage)
            bfQp = ctx.enter_context(tc.tile_pool(name="bfQ", bufs=3))
            # pre-rope'd y-keys borrow QT head-0 space: the deferred Y
            # transposes (Q hook, chunk 1) read it before Q's first
            # transpose drain (chunk 3) overwrites that region
            ykbf = QT[:, 0, :].rearrange("p (h f) -> p h f", h=KVH)
            lny = tc.alloc_tile_pool(name="lny", bufs=1)
            wY = tc.alloc_tile_pool(name="wY", bufs=1)
            yt = wY.tile([P, YDC, YL], BF, tag="yt")
            nc.sync.dma_start(out=yt, in_=y_d[:, :, :])
            wy_sb = []
            for g in range(YDC):
                wt = wY.tile([P, 2 * KVD], BF, tag=f"wy{g}", name=f"wy{g}")
                nc.sync.dma_start(out=wt, in_=wkvy_d[g])
                wy_sb.append(wt)
            kyw_t = bcast_vec(wY, kyw_d, KVD)
            kyb_t = bcast_vec(wY, kyb_d, KVD)
            psY = tc.alloc_tile_pool(name="psY", bufs=1, space="PSUM")
            ya = [psY.tile([P, NQ], F32, tag=f"ya{n}", bufs=1, name=f"ya{n}")
                  for n in range(4)]
            for dc in range(YDC):
                for n in range(4):
                    nc.tensor.matmul(
                        ya[n][:], lhsT=yt[:, dc, :],
                        rhs=wy_sb[dc][:, n * NQ:(n + 1) * NQ],
                        start=(dc == 0), stop=(dc == YDC - 1))
            ykn = wY.tile([P, KVD], F32, tag="ykn")
            for n in range(2):
                nc.scalar.copy(out=ykn[:, n * NQ:(n + 1) * NQ], in_=ya[n][:])
                nc.scalar.copy(out=YV[:, 4 * n:4 * (n + 1), :],
                               in_=ya[2 + n][:])
            ln_chain(ykn, 2, wY, kyw_t, kyb_t, "y")
            nc.gpsimd.tensor_mul(out=ykn, in0=ykn, in1=kyw_t)
            nc.gpsimd.tensor_add(out=ykn, in0=ykn, in1=kyb_t)
            nc.vector.tensor_copy(out=ykbf, in_=ykn)
            # Y's head transposes are deferred into the Q stage (hook below)
            # so they don't block Q's first matmuls behind the Y LN chain
            psY.release()
            wY.release()
            lny.release()

            # =========================================================
            # shared projection-stage machinery
            # =========================================================
            # final-contraction-round matmul order / copy order: the
            # single-buffered accumulators (2, 3) finish and copy first
            ACC_ORDER = [2, 3, 0, 1]

            def proj_stage(nchunks, x_dram, w_tiles, out_heads,
                           w_t, b_t, cos_dram, sin_dram, dst_T, dst_V, stage,
                           xs, preloaded, hooks, keep_last=0, bfp_ext=None):
                """One pass over `nchunks` token chunks with 4 accumulators.

                fp8 hi/lo path: `x_dram` is an (xh_d, xl_d) pair, `w_tiles` a
                list of DC//2 (w_hi, w_lo) pair-tiles [P, 2, nfeat]. Each
                accumulator gets 3 DoubleRow terms per dc pair (hh, hl, lh);
                all three share one PSUM scale since lo parts are unscaled.

                The first `out_heads*HD` features get LN+rope+transpose into
                dst_T; for the KV stage accumulators [2,3] are V, copied
                (restoring the 1/WSC weight pre-scale) into dst_V[:, t, :].
                `xs`: caller-owned x-tile pool (chunks in `preloaded` were
                DMA'd by the caller before the weight DMAs). `hooks[t]` runs
                after chunk t's x DMA — used to interleave next-stage
                prefetch DMAs into the SP queue.
                """
                nacc = 4
                xh_dram, xl_dram = x_dram
                npair = DC // 2
                bfp = bfp_ext or tc.alloc_tile_pool(name=f"bf{stage}",
                                                    bufs=4)
                csp = tc.alloc_tile_pool(name=f"cs{stage}", bufs=1)
                cs_tiles = {}
                nhalves = (nchunks + 7) // 8

                def load_cs_half(hh):
                    ct = csp.tile([P, 8, HD // 2], BF, tag="ctab",
                                  bufs=nhalves, name=f"ctab{stage}_{hh}")
                    st = csp.tile([P, 8, HD // 2], BF, tag="stab",
                                  bufs=nhalves, name=f"stab{stage}_{hh}")
                    nc.sync.dma_start(out=ct,
                                      in_=cos_dram[:, 8 * hh:8 * hh + 8, :])
                    nc.sync.dma_start(out=st,
                                      in_=sin_dram[:, 8 * hh:8 * hh + 8, :])
                    cs_tiles[hh] = (ct, st)

                load_cs_half(0)
                wk_ = tc.alloc_tile_pool(name=f"work{stage}", bufs=2)
                stp = tc.alloc_tile_pool(name=f"st{stage}", bufs=2)
                rtp = tc.alloc_tile_pool(name=f"rt{stage}", bufs=1)
                psP = tc.alloc_tile_pool(name=f"ps{stage}", bufs=1,
                                         space="PSUM")
                nfeat = out_heads * HD
                nln = nfeat // NQ        # accumulators covered by LN
                oh2 = out_heads // 2     # heads per half
                pending = []             # [(zbf, tok0)]

                def emit_transposes(zbf, tok0):
                    for hg in range(out_heads // 4):
                        h0 = hg * 4
                        tp = psP.tile([P, 4, P], BF, tag="tr", bufs=2)
                        for j in range(4):
                            nc.tensor.transpose(
                                tp[:, j, :], zbf[:, h0 + j, :], ident)
                        nc.scalar.copy(
                            out=dst_T[:, h0:h0 + 4, tok0:tok0 + P],
                            in_=tp)

                def drain(n):
                    for _ in range(n):
                        if pending:
                            emit_transposes(*pending.pop(0))

                for t in range(nchunks):
                    if t in preloaded:
                        xth, xtl = preloaded[t]
                    else:
                        xth = xs.tile([P, DC, P], F8, tag="xth",
                                      name=f"xth{stage}_{t}")
                        nc.sync.dma_start(out=xth, in_=xh_dram[t])
                        xtl = xs.tile([P, DC, P], F8, tag="xtl",
                                      name=f"xtl{stage}_{t}")
                        nc.sync.dma_start(out=xtl, in_=xl_dram[t])
                    if t in hooks:
                        hooks[t](psP)
                    if (t % 8 == 6 and t + 2 < nchunks
                            and (t + 2) // 8 not in cs_tiles):
                        load_cs_half((t + 2) // 8)
                    accs = [psP.tile([P, NQ], F32, tag=f"acc{n}",
                                     bufs=(2 if n < 2 else 1),
                                     name=f"acc{n}_{t}")
                            for n in range(nacc)]
                    # 3 hi/lo terms x npair DoubleRow rounds; last round in
                    # ACC_ORDER so single-buffered accs finish+copy first
                    rounds = []
                    for i in range(npair):
                        rounds.append((xth[:, 2 * i:2 * i + 2, :],
                                       w_tiles[i][0]))
                        rounds.append((xth[:, 2 * i:2 * i + 2, :],
                                       w_tiles[i][1]))
                        rounds.append((xtl[:, 2 * i:2 * i + 2, :],
                                       w_tiles[i][0]))
                    for r, (lt, wt) in enumerate(rounds[:-1]):
                        for n in range(nacc):
                            nc.tensor.matmul(
                                accs[n][:], lhsT=lt,
                                rhs=wt[:, :, n * NQ:(n + 1) * NQ],
                                start=(r == 0), stop=False, perf_mode=DR)
                    lt, wt = rounds[-1]
                    for n in ACC_ORDER:
                        nc.tensor.matmul(
                            accs[n][:], lhsT=lt,
                            rhs=wt[:, :, n * NQ:(n + 1) * NQ],
                            start=False, stop=True, perf_mode=DR)
                    # PSUM -> SBUF copies, staggered order matching the
                    # final round so the next chunk never waits
                    zn = wk_.tile([P, nfeat], BF, tag="work")
                    for n in ACC_ORDER:
                        if n < nln:
                            nc.scalar.copy(out=zn[:, n * NQ:(n + 1) * NQ],
                                           in_=accs[n][:])
                        elif dst_V is not None:
                            nc.scalar.activation(
                                out=dst_V[:, t, (n - nln) * NQ:
                                          (n - nln + 1) * NQ],
                                in_=accs[n][:], func=AF.Copy,
                                scale=1.0 / WSC)
                    # transposes of chunk t-3; their PSUM-drain copies ride
                    # on DVE, which is idle during this chunk's matmuls
                    if t >= 2:
                        drain(1)
                    ln_chain(zn, nln, stp, w_t, b_t, stage, eps=eps_s_t)
                    # affine on Pool + rope on DVE, split into head groups.
                    # The last two chunks split finer: their transposes are
                    # close to the stage tail, so a shorter serial chain
                    # (affine part -> rope part pipelined across engines)
                    # directly shortens the stage-exit stall.
                    nsplit = 4 if t >= nchunks - 2 else 2
                    ohs = out_heads // nsplit
                    zbf = bfp.tile([P, out_heads, HD], BF, tag="zbf")
                    zv = zn.rearrange("p (h f two) -> p h f two",
                                      h=out_heads, two=2)
                    zb = zbf.rearrange("p h (f two) -> p h f two", two=2)
                    ct_t, st_t = cs_tiles[t // 8]
                    shp = (P, ohs, HD // 2)
                    cb = ct_t[:, t % 8, :][:, None, :].to_broadcast(shp)
                    sb = st_t[:, t % 8, :][:, None, :].to_broadcast(shp)
                    for part in range(nsplit):
                        f0 = part * (nfeat // nsplit)
                        f1 = (part + 1) * (nfeat // nsplit)
                        nc.gpsimd.tensor_mul(out=zn[:, f0:f1],
                                             in0=zn[:, f0:f1],
                                             in1=w_t[:, f0:f1])
                        nc.gpsimd.tensor_add(out=zn[:, f0:f1],
                                             in0=zn[:, f0:f1],
                                             in1=b_t[:, f0:f1])
                        h0, h1 = part * ohs, (part + 1) * ohs
                        re = zv[:, h0:h1, :, 0]
                        im = zv[:, h0:h1, :, 1]
                        rebf = zb[:, h0:h1, :, 0]
                        imbf = zb[:, h0:h1, :, 1]
                        t1 = rtp.tile([P, ohs, HD // 2], BF, tag="r1")
                        t2 = rtp.tile([P, ohs, HD // 2], BF, tag="r2")
                        nc.vector.tensor_mul(out=t1, in0=re, in1=cb)
                        nc.vector.tensor_mul(out=t2, in0=im, in1=sb)
                        nc.vector.tensor_sub(out=rebf, in0=t1, in1=t2)
                        nc.vector.tensor_mul(out=t1, in0=re, in1=sb)
                        nc.vector.tensor_mul(out=t2, in0=im, in1=cb)
                        nc.vector.tensor_add(out=imbf, in0=t1, in1=t2)
                    pending.append((zbf, t * P))
                while len(pending) > keep_last:
                    drain(1)
                for pool in (psP, rtp, stp, wk_, csp):
                    pool.release()
                if keep_last == 0:
                    if bfp_ext is None:
                        bfp.release()
                    return [], None
                # bfp (bottom of this stage's transient stack) stays alive
                # so the caller can emit the kept chunks' transposes later
                return pending, bfp

            # =========================================================
            # Stage Q: local-half Q projection
            # =========================================================
            lnq = tc.alloc_tile_pool(name="lnq", bufs=1)
            qw_t = bcast_vec(lnq, qw_d, D)
            qb_t = bcast_vec(lnq, qb_d, D)
            xsQ = tc.alloc_tile_pool(name="xsQ", bufs=2)
            xq0h = xsQ.tile([P, DC, P], F8, tag="xth", name="xthQ_0")
            nc.sync.dma_start(out=xq0h, in_=xqh_d[0])
            xq0l = xsQ.tile([P, DC, P], F8, tag="xtl", name="xtlQ_0")
            nc.sync.dma_start(out=xq0l, in_=xql_d[0])
            xq1h = xsQ.tile([P, DC, P], F8, tag="xth", name="xthQ_1")
            nc.sync.dma_start(out=xq1h, in_=xqh_d[1])
            xq1l = xsQ.tile([P, DC, P], F8, tag="xtl", name="xtlQ_1")
            nc.sync.dma_start(out=xq1l, in_=xql_d[1])
            wQ = tc.alloc_tile_pool(name="wQ", bufs=1)
            wq_sb = []
            for g in range(DC // 2):
                wth = wQ.tile([P, 2, D], F8, tag=f"wqh{g}", name=f"wqh{g}")
                nc.sync.dma_start(out=wth, in_=wqh_d[g])
                wtl = wQ.tile([P, 2, D], F8, tag=f"wql{g}", name=f"wql{g}")
                nc.sync.dma_start(out=wtl, in_=wql_d[g])
                wq_sb.append((wth, wtl))

            # prefetch first half of wkv into untouched right-side SBUF
            wKVa = tc.alloc_tile_pool(name="wKVa", bufs=1, side="right")
            wkv_sb = [None] * (DC // 2)

            def hook_ytr(psP):
                for hg in range(2):
                    tp = psP.tile([P, 4, P], BF, tag="tr", bufs=2,
                                  name=f"ytr{hg}")
                    for j in range(4):
                        nc.tensor.transpose(
                            tp[:, j, :], ykbf[:, hg * 4 + j, :], ident)
                    nc.scalar.copy(
                        out=YKT[:, hg * 4:(hg + 1) * 4, :], in_=tp)

            def hook_wkva(psP):
                for g in range(4):
                    wth = wKVa.tile([P, 2, 2 * KVD], F8, tag=f"wkvh{g}",
                                    name=f"wkvh{g}")
                    nc.sync.dma_start(out=wth, in_=wkvh_d[g])
                    wtl = wKVa.tile([P, 2, 2 * KVD], F8, tag=f"wkvl{g}",
                                    name=f"wkvl{g}")
                    nc.sync.dma_start(out=wtl, in_=wkvl_d[g])
                    wkv_sb[g] = (wth, wtl)

            q_pending, _ = proj_stage(
                TCL, (xqh_d, xql_d), wq_sb, H, qw_t, qb_t,
                cosq_d, sinq_d, QT, None, "Q",
                xsQ, {0: (xq0h, xq0l), 1: (xq1h, xq1l)},
                {1: hook_ytr, 3: hook_wkva}, keep_last=2, bfp_ext=bfQp)
            wQ.release()
            xsQ.release()
            lnq.release()

            # =========================================================
            # Stage KV: full-seq K (LN+rope) and V projections, one x pass
            # =========================================================
            ktvp = ctx.enter_context(tc.tile_pool(name="ktvpool", bufs=1))
            KT = ktvp.tile([P, KVH, S], BF)
            Vsb = ktvp.tile([P, TC, KVD], BF)
            lnk = tc.alloc_tile_pool(name="lnk", bufs=1)
            kw_t = bcast_vec(lnk, kw_d, KVD)
            kb_t = bcast_vec(lnk, kb_d, KVD)
            # x tiles ahead of the wkv-second-half DMAs in the SP queue
            xsK = tc.alloc_tile_pool(name="xsK", bufs=2)
            xk0h = xsK.tile([P, DC, P], F8, tag="xth", name="xthK_0")
            nc.sync.dma_start(out=xk0h, in_=xh_d[0])
            xk0l = xsK.tile([P, DC, P], F8, tag="xtl", name="xtlK_0")
            nc.sync.dma_start(out=xk0l, in_=xl_d[0])
            xk1h = xsK.tile([P, DC, P], F8, tag="xth", name="xthK_1")
            nc.sync.dma_start(out=xk1h, in_=xh_d[1])
            xk1l = xsK.tile([P, DC, P], F8, tag="xtl", name="xtlK_1")
            nc.sync.dma_start(out=xk1l, in_=xl_d[1])
            # second wkv half into fresh right-side space: its DMAs have no
            # space-dependency on the Q stage and start immediately
            wKVb = tc.alloc_tile_pool(name="wKVb", bufs=1, side="right")
            for g in range(4, DC // 2):
                wth = wKVb.tile([P, 2, 2 * KVD], F8, tag=f"wkvh{g}",
                                name=f"wkvh{g}")
                nc.sync.dma_start(out=wth, in_=wkvh_d[g])
                wtl = wKVb.tile([P, 2, 2 * KVD], F8, tag=f"wkvl{g}",
                                name=f"wkvl{g}")
                nc.sync.dma_start(out=wtl, in_=wkvl_d[g])
                wkv_sb[g] = (wth, wtl)
            # Q's last two chunks' transposes ride in the KV stage (chunks
            # 1-2) so KV's first matmuls never sit behind Q's rope tail on
            # the in-order PE queue
            def hook_qtr(i):
                def run(psP):
                    zbf, tok0 = q_pending[i]
                    for hg in range(H // 4):
                        tp = psP.tile([P, 4, P], BF, tag="tr", bufs=2,
                                      name=f"qtr{i}_{hg}")
                        for j in range(4):
                            nc.tensor.transpose(
                                tp[:, j, :], zbf[:, hg * 4 + j, :], ident)
                        nc.scalar.copy(
                            out=QT[:, hg * 4:hg * 4 + 4, tok0:tok0 + P],
                            in_=tp)
                return run

            kv_pending, bfK = proj_stage(TC, (xh_d, xl_d), wkv_sb, KVH,
                                         kw_t, kb_t,
                                         cosk_d, sink_d, KT, Vsb, "K",
                                         xsK, {0: (xk0h, xk0l),
                                               1: (xk1h, xk1l)}, keep_last=3,
                                         hooks={1: hook_qtr(0),
                                                2: hook_qtr(1)})
            wKVb.release()
            wKVa.release()

            # =========================================================
            # Stage attention: per (head, q-chunk)
            # =========================================================
            mgp = tc.alloc_tile_pool(name="merged", bufs=1)
            merged_h = mgp.tile([P, H, S_LOC], F8)
            merged_l = mgp.tile([P, H, S_LOC], F8)
            wop = tc.alloc_tile_pool(name="wop", bufs=2)
            ep = tc.alloc_tile_pool(name="epool", bufs=4)
            esp = tc.alloc_tile_pool(name="espool", bufs=2)
            eyp = tc.alloc_tile_pool(name="eypool", bufs=2)
            rcp = tc.alloc_tile_pool(name="rcpool", bufs=1)
            psA = tc.alloc_tile_pool(name="psA", bufs=1, space="PSUM")
            outp = tc.alloc_tile_pool(name="outp", bufs=2)
            # prefetch first wo slices during attention
            wo_tiles = {}
            for oc in range(2):
                wo_th = wop.tile([P, DC, P], F8, tag="woh", name=f"woh{oc}")
                nc.sync.dma_start(out=wo_th, in_=woh_d[oc])
                wo_tl = wop.tile([P, DC, P], F8, tag="wol", name=f"wol{oc}")
                nc.sync.dma_start(out=wo_tl, in_=wol_d[oc])
                wo_tiles[oc] = (wo_th, wo_tl)

            def emit_outproj_block(oc, qc):
                """24 DoubleRow matmuls into an o-ring PSUM slot + DVE
                scaled copy + DMA out. ACT-free so it absorbs the exp
                backlog when interleaved."""
                if oc in wo_tiles:
                    wo_th, wo_tl = wo_tiles.pop(oc)
                else:
                    wo_th = wop.tile([P, DC, P], F8, tag="woh",
                                     name=f"woh{oc}_{qc}")
                    nc.sync.dma_start(out=wo_th, in_=woh_d[oc])
                    wo_tl = wop.tile([P, DC, P], F8, tag="wol",
                                     name=f"wol{oc}_{qc}")
                    nc.sync.dma_start(out=wo_tl, in_=wol_d[oc])
                q0 = qc * NQ
                out_ps = psA.tile([P, NQ], F32, tag="o", bufs=2,
                                  name=f"ops{oc}_{qc}")
                rounds = []
                for i in range(DC // 2):
                    sl = slice(2 * i, 2 * i + 2)
                    rounds.append((wo_th[:, sl, :], merged_h[:, sl,
                                                            q0:q0 + NQ]))
                    rounds.append((wo_tl[:, sl, :], merged_h[:, sl,
                                                            q0:q0 + NQ]))
                    rounds.append((wo_th[:, sl, :], merged_l[:, sl,
                                                            q0:q0 + NQ]))
                for r, (lt, rt) in enumerate(rounds):
                    nc.tensor.matmul(
                        out_ps[:], lhsT=lt, rhs=rt,
                        start=(r == 0), stop=(r == len(rounds) - 1),
                        perf_mode=DR)
                out_t = outp.tile([P, NQ], F32, tag="outt")
                nc.vector.tensor_scalar_mul(out_t, out_ps[:],
                                            1.0 / (16.0 * WSC))
                nc.sync.dma_start(
                    out=outT[oc * P:(oc + 1) * P, q0:q0 + NQ],
                    in_=out_t)

            # cross scores for iteration idx+1 are issued at g5 of idx so
            # ACT's Ey exp never waits on a cold sy matmul
            def emit_sy(idx):
                qc, h = divmod(idx, H)
                kv = h // NREP
                sy = psA.tile([P, NQ], F32, tag="cr", bufs=1,
                              name=f"sy{idx}")
                nc.tensor.matmul(sy[:], lhsT=YKT[:, kv, :],
                                 rhs=QT[:, h, qc * NQ:qc * NQ + NQ],
                                 start=True, stop=True,
                                 skip_group_check=True)
                Ey = eyp.tile([P, NQ], BF, tag="Ey", name=f"Ey{idx}")
                nc.scalar.activation(out=Ey, in_=sy[:], func=AF.Exp,
                                     scale=SCALE, bias=ymb_t)
                return Ey

            def emit_tail(st):
                """Denominator tail + merge of the previous iteration,
                emitted after the next iteration's first score group so the
                exp backlog never blocks the PE at iteration boundaries."""
                es8_, d_, o_, t1_, h_, q0_ = st
                nc.tensor.matmul(d_[:], lhsT=ones_t, rhs=es8_[:, 2, :],
                                 start=False, stop=False,
                                 skip_group_check=True)
                nc.tensor.matmul(d_[:], lhsT=ones_t, rhs=es8_[:, 3, :],
                                 start=False, stop=False,
                                 skip_group_check=True)
                nc.tensor.matmul(d_[:], lhsT=ones_t, rhs=es8_[:, 4, :],
                                 start=False, stop=True,
                                 skip_group_check=True)
                rec = rcp.tile([P, NQ], F32, tag="rec")
                nc.vector.reciprocal(out=rec, in_=d_[:])
                t0 = rcp.tile([P, NQ], F32, tag="t0")
                nc.vector.tensor_mul(out=t0, in0=o_[:], in1=rec)
                # merged (x16 via the 1/16 ones) as fp8 hi + lo residual;
                # t0 doubles as the f32 sum (in-place add)
                nc.gpsimd.tensor_add(out=t0, in0=t0, in1=t1_)
                nc.gpsimd.tensor_copy(out=merged_h[:, h_, q0_:q0_ + NQ],
                                      in_=t0)
                nc.gpsimd.tensor_sub(out=merged_l[:, h_, q0_:q0_ + NQ],
                                     in0=t0,
                                     in1=merged_h[:, h_, q0_:q0_ + NQ])

            NIT = QCN * H
            Ey = emit_sy(0)
            deferred = None
            for idx in range(NIT):
                qc, h = divmod(idx, H)
                kv = h // NREP
                q0 = qc * NQ
                qt = QT[:, h, q0:q0 + NQ]
                o_ps = psA.tile([P, NQ], F32, tag="o", bufs=2)
                d_ps = None
                # compact 5-slot sum ring: g->slot [0,1,1,2,2,3,3,4]; the in-place
                # tree (g1,3,5) folds pairs into slots 0-2, freeing 3 slots
                es8 = esp.tile([P, 5, NQ], BF, tag="es8")

                # AV rides one group behind the scores: while ACT exps
                # group g, the PE runs group g+1's score matmuls instead of
                # head-of-line-blocking on AV(g); AV(g) lands after them.
                def emit_av(E_, g_):
                    for j in range(2):
                        kc = 2 * g_ + j
                        nc.tensor.matmul(
                            o_ps[:],
                            lhsT=Vsb[:, kc, kv * HD:(kv + 1) * HD],
                            rhs=E_[:, j, :],
                            start=(kc == 0), stop=(kc == TC - 1),
                            skip_group_check=True)

                prev_E = None
                for g in range(8):
                    if idx == 0 and g >= 5 and kv_pending:
                        # chunks 14/15's K transposes, deferred past their
                        # rope chains; first consumer is g7's score mms
                        zbf15, ktok = kv_pending.pop(0)
                        for hg in range(2):
                            tp = psA.tile([P, 4, P], BF, tag="s", bufs=2,
                                          name=f"ktr{ktok}_{hg}")
                            for j in range(4):
                                nc.tensor.transpose(
                                    tp[:, j, :], zbf15[:, hg * 4 + j, :],
                                    ident)
                            nc.scalar.copy(
                                out=KT[:, hg * 4:(hg + 1) * 4,
                                       ktok:ktok + P],
                                in_=tp)
                    s_ps = psA.tile([P, 2, NQ], F32, tag="s", bufs=2)
                    for j in range(2):
                        kc = 2 * g + j
                        nc.tensor.matmul(
                            s_ps[:, j, :],
                            lhsT=KT[:, kv, kc * P:(kc + 1) * P],
                            rhs=qt, start=True, stop=True,
                            skip_group_check=True)
                    if prev_E is not None:
                        emit_av(prev_E, g - 1)
                    E = ep.tile([P, 2, NQ], BF, tag="E")
                    nc.scalar.activation(out=E, in_=s_ps[:],
                                         func=AF.Exp, scale=SCALE)
                    nc.vector.tensor_add(
                        out=es8[:, (g + 1) // 2, :],
                        in0=E[:, 0, :], in1=E[:, 1, :])
                    prev_E = E
                    if g == 0 and deferred is not None:
                        emit_tail(deferred)
                        deferred = None
                    if g in (1, 3, 5):
                        # lvl1 tree add on Pool (in-place into es8[0:3])
                        i = g // 2
                        nc.gpsimd.tensor_add(out=es8[:, i, :],
                                             in0=es8[:, i, :],
                                             in1=es8[:, i + 1, :])
                    if g == 1:
                        dy = psA.tile([P, NQ], F32, tag="cr", bufs=1,
                                      name=f"dy{idx}")
                        nc.tensor.matmul(dy[:], lhsT=ones_t, rhs=Ey,
                                         start=True, stop=True,
                                         skip_group_check=True)
                        rec_y = rcp.tile([P, NQ], F32, tag="recy")
                        nc.vector.reciprocal(out=rec_y, in_=dy[:])
                    elif g == 3:
                        d_ps = psA.tile([P, NQ], F32, tag="d", bufs=1,
                                        name=f"d{idx}")
                        nc.tensor.matmul(
                            d_ps[:], lhsT=ones_t, rhs=es8[:, 0, :],
                            start=True, stop=False,
                            skip_group_check=True)
                        oy = psA.tile([P, NQ], F32, tag="cr", bufs=1,
                                      name=f"oy{idx}")
                        nc.tensor.matmul(oy[:], lhsT=YV[:, kv, :],
                                         rhs=Ey, start=True, stop=True,
                                         skip_group_check=True)
                        t1 = rcp.tile([P, NQ], F32, tag="t1", bufs=2)
                        nc.vector.scalar_tensor_tensor(
                            out=t1, in0=oy[:],
                            scalar=gates_t[:, h:h + 1],
                            in1=rec_y, op0=ALU.mult, op1=ALU.mult)
                    elif g == 5:
                        nc.tensor.matmul(
                            d_ps[:], lhsT=ones_t, rhs=es8[:, 1, :],
                            start=False, stop=False,
                            skip_group_check=True)
                        if idx + 1 < NIT:
                            next_Ey = emit_sy(idx + 1)

                emit_av(prev_E, 7)
                deferred = (es8, d_ps, o_ps, t1, h, q0)
                Ey = next_Ey
                # during the second q-chunk pass, interleave the first
                # q-chunk's output projection (ACT-free PE work)
                if qc == 1:
                    emit_outproj_block(h, 0)

            if deferred is not None:
                emit_tail(deferred)
                deferred = None
            # remaining out-proj: second q-chunk
            for oc in range(DC):
                emit_outproj_block(oc, 1)
            psA.release()
            outp.release()
            rcp.release()
            eyp.release()
            esp.release()
            ep.release()
            wop.release()
            mgp.release()
            bfK.release()
            xsK.release()
            lnk.release()

    _split_dma_waits(nc)
    return nc


def _prep_inputs(x, y, freqs_cos, freqs_sin, y_mask, wq, wk, wv, wk_y, wv_y,
                 wo, q_w, q_b, k_w, k_b, ky_w, ky_b, gate):
    f32 = np.float32

    def hilo(v):
        hi = v.astype(F8NP)
        lo = (v - hi.astype(f32)).astype(F8NP)
        return hi, lo

    def chunk_x(xb):
        # [S, D] -> [tc, p, dc, s]: out[t, p, dc, s] = xb[t*128+s, dc*128+p]
        # as an fp8 (hi, lo) pair
        t = xb.shape[0] // P
        ch = np.ascontiguousarray(
            xb.reshape(t, P, DC, P).transpose(0, 3, 2, 1))
        return hilo(ch)

    def pair_w(wm):
        # [D, F] -> fp8 hi/lo pairs [DC/2, p, 2, F] (WSC pre-scale)
        fdim = wm.shape[1]
        ws = np.ascontiguousarray(
            (np.asarray(wm, f32) * WSC).reshape(DC // 2, 2, P, fdim)
            .transpose(0, 2, 1, 3))
        return hilo(ws)

    def chunk_cs(tab):
        # [S', 64] -> [p, t, f]
        t = tab.shape[0] // P
        return np.ascontiguousarray(
            np.asarray(tab, f32).reshape(t, P, HD // 2).transpose(1, 0, 2)
            .astype(BF16))

    wo_s = np.ascontiguousarray(
        (np.asarray(wo, f32) * WSC).reshape(DC, P, DC, P)
        .transpose(2, 1, 0, 3))
    woh8 = wo_s.astype(F8NP)
    wol8 = (wo_s - woh8.astype(f32)).astype(F8NP)
    wqh, wql = pair_w(np.asarray(wq, f32))
    wkvh, wkvl = pair_w(np.concatenate(
        [np.asarray(wk, f32), np.asarray(wv, f32)], axis=1))
    shared = {
        "wqh": wqh,
        "wql": wql,
        "wkvh": wkvh,
        "wkvl": wkvl,
        "wkvy": np.ascontiguousarray(np.concatenate(
            [np.asarray(wk_y, f32), np.asarray(wv_y, f32)],
            axis=1).astype(BF16).reshape(YDC, P, 2 * KVD)),
        "woh": woh8,
        "wol": wol8,
        "qw": np.ascontiguousarray(np.asarray(q_w, f32).astype(BF16)),
        "qb": np.ascontiguousarray(np.asarray(q_b, f32).astype(BF16)),
        "kw": np.ascontiguousarray(np.asarray(k_w, f32).astype(BF16)),
        "kb": np.ascontiguousarray(np.asarray(k_b, f32).astype(BF16)),
        "kyw": np.ascontiguousarray(np.asarray(ky_w, f32).astype(BF16)),
        "kyb": np.ascontiguousarray(np.asarray(ky_b, f32).astype(BF16)),
        "cosk": chunk_cs(freqs_cos),
        "sink": chunk_cs(freqs_sin),
        "gates": np.ascontiguousarray(np.tanh(np.asarray(gate, f32))),
    }
    per_core = []
    for c in range(8):
        b, hf = c // 2, c % 2
        sl = slice(hf * S_LOC, (hf + 1) * S_LOC)
        xb = np.asarray(x[b], f32)
        m = dict(shared)
        m["xh"], m["xl"] = chunk_x(xb)
        m["xqh"], m["xql"] = chunk_x(xb[sl])
        m["y"] = np.ascontiguousarray(
            np.asarray(y[b], f32).T.astype(BF16).reshape(YDC, P, YL)
            .transpose(1, 0, 2))
        m["cosq"] = chunk_cs(np.asarray(freqs_cos, f32)[sl])
        m["sinq"] = chunk_cs(np.asarray(freqs_sin, f32)[sl])
        m["ymb"] = np.where(np.asarray(y_mask[b]), 0.0, -1e9).astype(f32)
        per_core.append(m)
    return per_core


def kernel(**inputs):
    if "nc" not in _CACHED:
        _CACHED["nc"] = build_program()
    nc = _CACHED["nc"]
    in_maps = _prep_inputs(
        inputs["x"], inputs["y"], inputs["freqs_cos"], inputs["freqs_sin"],
        inputs["y_mask"], inputs["wq"], inputs["wk"], inputs["wv"],
        inputs["wk_y"], inputs["wv_y"], inputs["wo"], inputs["q_w"],
        inputs["q_b"], inputs["k_w"], inputs["k_b"], inputs["ky_w"],
        inputs["ky_b"], inputs["gate"])
    res = run_bass_kernel_spmd(nc, in_maps, core_ids=list(range(8)))
    global LAST_EXEC_NS
    LAST_EXEC_NS = res.exec_time_ns
    out = np.zeros((B, S, D), np.float32)
    for c in range(8):
        b, hf = c // 2, c % 2
        out[b, hf * S_LOC:(hf + 1) * S_LOC, :] = res.results[c]["outT"].T
    return out


if __name__ == "__main__":
    nc = build_program()
    print("program built OK")



# revision 59
# speedup vs baseline: 1.0223x; 1.0017x over previous
"""Trainium2 Bass kernel for nn_Attention_35734127903400 (v3).

Dense transformer attention block:
  xq = LN(x@wq); xk = LN(x@wk); xv = x@wv          (LN over full flattened head dim)
  rope(q, k); GQA self-attention (16 q heads, 8 kv heads, S=2048, full/non-causal)
  gated cross-attention with y (128 tokens); out = (self + tanh(gate)*cross) @ wo

Sharding (8 cores, no collectives): token-sharded. Core c handles batch
b=c//2, sequence half hf=c%2 (1024 q tokens). Each core computes K/V for
its batch's FULL 2048-token sequence (replicated within the pair), Q only
for its local 1024 tokens. LN is over the feature dim so it is core-local.

v3: the Q/KV/out projections run as fp8e4m3 hi+lo 3-term matmuls in
DoubleRow perf mode (2 contraction k-tiles per instruction at 0.5
cycles/row): x = hi + lo with hi = fp8(x), lo = fp8(x - hi), and
x@w ~= xh@wh + xh@wl + xl@wh. Validated offline at bf16-equivalent
accuracy (the dropped lo*lo term is ~0.1%). Weights pre-scale by WSC=64
into fp8's normal range; the Q/K LN absorbs the scale (eps scaled by
WSC^2), V restores it in its PSUM->SBUF copy. The softmax-path "ones"
matmul uses 1/16 so merged is built x16 for a well-scaled fp8 hi/lo
split feeding the out-proj; the final copy restores /(16*WSC).
Attention scores/AV/exp stay bf16/f32 — fp8 there fails the error
budget (softmax output averages signal down as fast as noise).

Everything stays resident in SBUF — no DRAM spill/reload. Stage order:
y proj (startup filler while wq streams) -> Q proj -> K+V proj (fused,
one x pass) -> attention -> output proj (wo streamed per output chunk).

Scheduling specifics, tuned against the TimelineSim cost model:
- proj chunks: the final contraction round runs the single-buffered
  accumulators first and the PSUM->SBUF copies are emitted in the same
  order, so the next chunk's matmuls never wait on a copy.
- LN affine (Pool) + rope (DVE) run per head-half; head transposes (PE,
  1 cycle/row) trail the rope by 1-3 chunks; the KV stage's last two
  chunks' transposes defer into attention idx 0 (g6/g7) so attention's
  first matmuls never sit behind the rope tail on the in-order PE queue.
- attention: softmax denominator via DVE/Pool pairwise-add tree feeding
  5 ones-matmuls spread through the AV stream; cross-attention matmuls
  (dy/oy) slotted where their PSUM bank is free; exp is the pacing
  engine (ACT); the per-iteration tail (denominator finish + merge)
  defers to g0 of the next iteration so it never delays the next score
  group's exps.
- weight DMAs are per-dc-pair so matmuls start after the first slice;
  wkv prefetches into an untouched right-side SBUF region during Q so
  the KV stage starts without a DMA bubble.
"""

import numpy as np
import ml_dtypes

import concourse.bass as bass
import concourse.mybir as mybir
import concourse.tile as tile
from concourse.bass_utils import run_bass_kernel_spmd
from concourse.masks import make_identity

BF16 = ml_dtypes.bfloat16
F8NP = ml_dtypes.float8_e4m3fn
F32 = mybir.dt.float32
BF = mybir.dt.bfloat16
F8 = mybir.dt.float8e4
WSC = 64.0        # weight pre-scale before fp8 split (w ~ N(0, .02))
DR = mybir.MatmulPerfMode.DoubleRow

P = 128
B, S, D = 4, 2048, 2048
H, KVH = 16, 8
HD = 128
NREP = 2
YL, YD = 128, 1024
EPS = 1e-5
S_LOC = S // 2
DC = D // P          # 16 contraction chunks for D
YDC = YD // P        # 8
TC = S // P          # 16 token chunks (full seq)
TCL = S_LOC // P     # 8 local token chunks
NQ = 512
QCN = S_LOC // NQ    # 2
KVD = KVH * HD       # 1024
SCALE = 1.0 / float(np.sqrt(np.float32(HD)))
AF = mybir.ActivationFunctionType
ALU = mybir.AluOpType

_CACHED = {}
LAST_EXEC_NS = None


def _split_dma_waits(nc, max_waits=1):
    """Hoist excess sync-waits of any instruction onto preceding same-engine
    single-wait NoOps (this build's per-instruction structs have few embedded
    wait slots)."""
    n_split = 0
    for f in nc.m.functions:
        for blk in f.blocks:
            insts = list(blk.instructions)
            out = []
            changed = False
            for ins in insts:
                si = ins.sync_info
                if (si is not None and si.on_wait
                        and len(si.on_wait) > max_waits):
                    waits = list(si.on_wait)
                    for wi, w in enumerate(waits[:-max_waits]):
                        out.append(mybir.InstNoOp(
                            name=f"{ins.name}-wsplit{wi}", engine=ins.engine,
                            sync_info=mybir.SyncInfo(on_wait=[w],
                                                     on_update=[])))
                    ins.sync_info = mybir.SyncInfo(
                        on_wait=waits[-max_waits:],
                        on_update=list(si.on_update))
                    changed = True
                    n_split += 1
                out.append(ins)
            if changed:
                blk.instructions = out
    return n_split


def build_program():
    nc = bass.Bass()

    # ---- I/O (all pre-chunked host-side for >=2KB contiguous runs) ----
    # x and the Q/KV weights ship as fp8 hi+lo pairs (hi = fp8(v),
    # lo = fp8(v - hi)); matmuls run 3-term hi*hi + hi*lo + lo*hi in
    # DoubleRow mode (2 contraction chunks per instruction, 0.5 cyc/row).
    # Weights are pre-scaled by WSC so they sit in fp8's normal range;
    # the Q/K layernorm absorbs the scale, V restores it in the PSUM copy.
    xqh_d = nc.declare_dram_parameter("xqh", [TCL, P, DC, P], F8,
                                      isOutput=False)
    xql_d = nc.declare_dram_parameter("xql", [TCL, P, DC, P], F8,
                                      isOutput=False)
    xh_d = nc.declare_dram_parameter("xh", [TC, P, DC, P], F8, isOutput=False)
    xl_d = nc.declare_dram_parameter("xl", [TC, P, DC, P], F8, isOutput=False)
    y_d = nc.declare_dram_parameter("y", [P, YDC, YL], BF, isOutput=False)
    wqh_d = nc.declare_dram_parameter("wqh", [DC // 2, P, 2, D], F8,
                                      isOutput=False)
    wql_d = nc.declare_dram_parameter("wql", [DC // 2, P, 2, D], F8,
                                      isOutput=False)
    wkvh_d = nc.declare_dram_parameter("wkvh", [DC // 2, P, 2, 2 * KVD], F8,
                                       isOutput=False)
    wkvl_d = nc.declare_dram_parameter("wkvl", [DC // 2, P, 2, 2 * KVD], F8,
                                       isOutput=False)
    wkvy_d = nc.declare_dram_parameter("wkvy", [YDC, P, 2 * KVD], BF,
                                       isOutput=False)
    woh_d = nc.declare_dram_parameter("woh", [DC, P, DC, P], F8,
                                      isOutput=False)
    wol_d = nc.declare_dram_parameter("wol", [DC, P, DC, P], F8,
                                      isOutput=False)
    qw_d = nc.declare_dram_parameter("qw", [D], BF, isOutput=False)
    qb_d = nc.declare_dram_parameter("qb", [D], BF, isOutput=False)
    kw_d = nc.declare_dram_parameter("kw", [KVD], BF, isOutput=False)
    kb_d = nc.declare_dram_parameter("kb", [KVD], BF, isOutput=False)
    kyw_d = nc.declare_dram_parameter("kyw", [KVD], BF, isOutput=False)
    kyb_d = nc.declare_dram_parameter("kyb", [KVD], BF, isOutput=False)
    cosq_d = nc.declare_dram_parameter("cosq", [P, TCL, HD // 2], BF,
                                       isOutput=False)
    sinq_d = nc.declare_dram_parameter("sinq", [P, TCL, HD // 2], BF,
                                       isOutput=False)
    cosk_d = nc.declare_dram_parameter("cosk", [P, TC, HD // 2], BF,
                                       isOutput=False)
    sink_d = nc.declare_dram_parameter("sink", [P, TC, HD // 2], BF,
                                       isOutput=False)
    gates_d = nc.declare_dram_parameter("gates", [H], F32, isOutput=False)
    ymb_d = nc.declare_dram_parameter("ymb", [YL], F32, isOutput=False)
    outT = nc.declare_dram_parameter("outT", [D, S_LOC], F32, isOutput=True)

    with tile.TileContext(nc) as tc:
        from contextlib import ExitStack
        with ExitStack() as ctx:
            cpool = ctx.enter_context(tc.tile_pool(name="consts", bufs=1))
            ident = cpool.tile([P, P], BF)
            make_identity(nc, ident)
            # 1/16 instead of 1.0: scales both softmax reciprocals by 16 so
            # merged is stored x16 (fp8-friendly range); the out-proj copy
            # restores /(16*WSC)
            ones_t = cpool.tile([P, P], BF)
            nc.vector.memset(ones_t, 1.0 / 16.0)
            eps_t = cpool.tile([P, 1], F32)
            nc.vector.memset(eps_t, EPS)
            # Q/K projections come out of the fp8 path scaled by WSC,
            # so their LN stats see var*WSC^2 — scale eps to match.
            eps_s_t = cpool.tile([P, 1], F32)
            nc.vector.memset(eps_s_t, EPS * WSC * WSC)
            gates_t = cpool.tile([P, H], F32)
            nc.gpsimd.dma_start(
                out=gates_t,
                in_=bass.AP(tensor=gates_d, offset=0, ap=[[0, P], [1, H]]))
            ymb_t = cpool.tile([P, 1], F32)
            nc.gpsimd.dma_start(
                out=ymb_t,
                in_=bass.AP(tensor=ymb_d, offset=0, ap=[[1, P], [0, 1]]))

            def bcast_vec(pool, dram_h, n):
                # bf16 vectors halve the stride-0 broadcast DMA bytes that
                # compete with the weight streams at stage starts (numerics:
                # systematic per-feature rounding, measured negligible)
                t = pool.tile([P, n], BF, tag=f"ln_{dram_h.name}", bufs=1)
                nc.gpsimd.dma_start(
                    out=t,
                    in_=bass.AP(tensor=dram_h, offset=0, ap=[[0, P], [1, n]]))
                return t

            def ln_chain(zn, nln, pool, w_t, b_t, pfx, eps=None):
                """stats (DVE) -> rstd/negmr -> normalize (ACT). Affine is
                applied by the caller (per-half on Pool)."""
                stats = pool.tile([P, nln, 6], F32, tag=f"{pfx}bnstats")
                for i in range(nln):
                    nc.vector.bn_stats(out=stats[:, i, :],
                                       in_=zn[:, i * NQ:(i + 1) * NQ])
                mv = pool.tile([P, 2], F32, tag=f"{pfx}bnaggr")
                nc.vector.bn_aggr(out=mv, in_=stats)
                rstd = pool.tile([P, 1], F32, tag=f"{pfx}rstd")
                nc.scalar.activation(out=rstd, in_=mv[:, 1:2],
                                     func=AF.Sqrt,
                                     bias=eps if eps is not None else eps_t,
                                     scale=1.0)
                nc.vector.reciprocal(out=rstd, in_=rstd)
                negmr = pool.tile([P, 1], F32, tag=f"{pfx}negmr")
                nc.vector.tensor_scalar(
                    out=negmr, in0=mv[:, 0:1], scalar1=rstd, scalar2=-1.0,
                    op0=ALU.mult, op1=ALU.mult)
                nc.scalar.activation(out=zn, in_=zn, func=AF.Identity,
                                     scale=rstd, bias=negmr)

            # =========================================================
            # Stage Y: y projections -> YKT (LN, no rope), YV.
            # Runs first: its matmuls fill the PE while wq streams in.
            # =========================================================
            yp = ctx.enter_context(tc.tile_pool(name="ypool", bufs=1))
            YKT = yp.tile([P, KVH, YL], BF)
            YV = yp.tile([P, KVH, HD], BF)
            qtp = ctx.enter_context(tc.tile_pool(name="qtpool", bufs=1))
            QT = qtp.tile([P, H, S_LOC], BF)
            # Q-stage rope-output ring lives low in the stack so it can
            # outlive wQ/xsQ (its last 2 chunks' transposes defer into the
            # KV stage)
            bfQp = ctx.enter_context(tc.tile_pool(name="bfQ", bufs=3))
            # pre-rope'd y-keys borrow QT head-0 space: the deferred Y
            # transposes (Q hook, chunk 1) read it before Q's first
            # transpose drain (chunk 3) overwrites that region
            ykbf = QT[:, 0, :].rearrange("p (h f) -> p h f", h=KVH)
            lny = tc.alloc_tile_pool(name="lny", bufs=1)
            wY = tc.alloc_tile_pool(name="wY", bufs=1)
            yt = wY.tile([P, YDC, YL], BF, tag="yt")
            nc.sync.dma_start(out=yt, in_=y_d[:, :, :])
            wy_sb = []
            for g in range(YDC):
                wt = wY.tile([P, 2 * KVD], BF, tag=f"wy{g}", name=f"wy{g}")
                nc.sync.dma_start(out=wt, in_=wkvy_d[g])
                wy_sb.append(wt)
            kyw_t = bcast_vec(wY, kyw_d, KVD)
            kyb_t = bcast_vec(wY, kyb_d, KVD)
            psY = tc.alloc_tile_pool(name="psY", bufs=1, space="PSUM")
            ya = [psY.tile([P, NQ], F32, tag=f"ya{n}", bufs=1, name=f"ya{n}")
                  for n in range(4)]
            for dc in range(YDC):
                for n in range(4):
                    nc.tensor.matmul(
                        ya[n][:], lhsT=yt[:, dc, :],
                        rhs=wy_sb[dc][:, n * NQ:(n + 1) * NQ],
                        start=(dc == 0), stop=(dc == YDC - 1))
            ykn = wY.tile([P, KVD], F32, tag="ykn")
            for n in range(2):
                nc.scalar.copy(out=ykn[:, n * NQ:(n + 1) * NQ], in_=ya[n][:])
                nc.scalar.copy(out=YV[:, 4 * n:4 * (n + 1), :],
                               in_=ya[2 + n][:])
            ln_chain(ykn, 2, wY, kyw_t, kyb_t, "y")
            nc.gpsimd.tensor_mul(out=ykn, in0=ykn, in1=kyw_t)
            nc.gpsimd.tensor_add(out=ykn, in0=ykn, in1=kyb_t)
            nc.vector.tensor_copy(out=ykbf, in_=ykn)
            # Y's head transposes are deferred into the Q stage (hook below)
            # so they don't block Q's first matmuls behind the Y LN chain
            psY.release()
            wY.release()
            lny.release()

            # =========================================================
            # shared projection-stage machinery
            # =========================================================
            # final-contraction-round matmul order / copy order: the
            # single-buffered accumulators (2, 3) finish and copy first
            ACC_ORDER = [2, 3, 0, 1]

            def proj_stage(nchunks, x_dram, w_tiles, out_heads,
                           w_t, b_t, cos_dram, sin_dram, dst_T, dst_V, stage,
                           xs, preloaded, hooks, keep_last=0, bfp_ext=None):
                """One pass over `nchunks` token chunks with 4 accumulators.

                fp8 hi/lo path: `x_dram` is an (xh_d, xl_d) pair, `w_tiles` a
                list of DC//2 (w_hi, w_lo) pair-tiles [P, 2, nfeat]. Each
                accumulator gets 3 DoubleRow terms per dc pair (hh, hl, lh);
                all three share one PSUM scale since lo parts are unscaled.

                The first `out_heads*HD` features get LN+rope+transpose into
                dst_T; for the KV stage accumulators [2,3] are V, copied
                (restoring the 1/WSC weight pre-scale) into dst_V[:, t, :].
                `xs`: caller-owned x-tile pool (chunks in `preloaded` were
                DMA'd by the caller before the weight DMAs). `hooks[t]` runs
                after chunk t's x DMA — used to interleave next-stage
                prefetch DMAs into the SP queue.
                """
                nacc = 4
                xh_dram, xl_dram = x_dram
                npair = DC // 2
                bfp = bfp_ext or tc.alloc_tile_pool(name=f"bf{stage}",
                                                    bufs=4)
                csp = tc.alloc_tile_pool(name=f"cs{stage}", bufs=1)
                cs_tiles = {}
                nhalves = (nchunks + 7) // 8

                def load_cs_half(hh):
                    ct = csp.tile([P, 8, HD // 2], BF, tag="ctab",
                                  bufs=nhalves, name=f"ctab{stage}_{hh}")
                    st = csp.tile([P, 8, HD // 2], BF, tag="stab",
                                  bufs=nhalves, name=f"stab{stage}_{hh}")
                    nc.sync.dma_start(out=ct,
                                      in_=cos_dram[:, 8 * hh:8 * hh + 8, :])
                    nc.sync.dma_start(out=st,
                                      in_=sin_dram[:, 8 * hh:8 * hh + 8, :])
                    cs_tiles[hh] = (ct, st)

                load_cs_half(0)
                wk_ = tc.alloc_tile_pool(name=f"work{stage}", bufs=2)
                stp = tc.alloc_tile_pool(name=f"st{stage}", bufs=2)
                rtp = tc.alloc_tile_pool(name=f"rt{stage}", bufs=1)
                psP = tc.alloc_tile_pool(name=f"ps{stage}", bufs=1,
                                         space="PSUM")
                nfeat = out_heads * HD
                nln = nfeat // NQ        # accumulators covered by LN
                oh2 = out_heads // 2     # heads per half
                pending = []             # [(zbf, tok0)]

                def emit_transposes(zbf, tok0):
                    for hg in range(out_heads // 4):
                        h0 = hg * 4
                        tp = psP.tile([P, 4, P], BF, tag="tr", bufs=2)
                        for j in range(4):
                            nc.tensor.transpose(
                                tp[:, j, :], zbf[:, h0 + j, :], ident)
                        nc.scalar.copy(
                            out=dst_T[:, h0:h0 + 4, tok0:tok0 + P],
                            in_=tp)

                def drain(n):
                    for _ in range(n):
                        if pending:
                            emit_transposes(*pending.pop(0))

                for t in range(nchunks):
                    if t in preloaded:
                        xth, xtl = preloaded[t]
                    else:
                        xth = xs.tile([P, DC, P], F8, tag="xth",
                                      name=f"xth{stage}_{t}")
                        nc.sync.dma_start(out=xth, in_=xh_dram[t])
                        xtl = xs.tile([P, DC, P], F8, tag="xtl",
                                      name=f"xtl{stage}_{t}")
                        nc.sync.dma_start(out=xtl, in_=xl_dram[t])
                    if t in hooks:
                        hooks[t](psP)
                    if (t % 8 == 6 and t + 2 < nchunks
                            and (t + 2) // 8 not in cs_tiles):
                        load_cs_half((t + 2) // 8)
                    accs = [psP.tile([P, NQ], F32, tag=f"acc{n}",
                                     bufs=(2 if n < 2 else 1),
                                     name=f"acc{n}_{t}")
                            for n in range(nacc)]
                    # 3 hi/lo terms x npair DoubleRow rounds; last round in
                    # ACC_ORDER so single-buffered accs finish+copy first
                    rounds = []
                    for i in range(npair):
                        rounds.append((xth[:, 2 * i:2 * i + 2, :],
                                       w_tiles[i][0]))
                        rounds.append((xth[:, 2 * i:2 * i + 2, :],
                                       w_tiles[i][1]))
                        rounds.append((xtl[:, 2 * i:2 * i + 2, :],
                                       w_tiles[i][0]))
                    for r, (lt, wt) in enumerate(rounds[:-1]):
                        for n in range(nacc):
                            nc.tensor.matmul(
                                accs[n][:], lhsT=lt,
                                rhs=wt[:, :, n * NQ:(n + 1) * NQ],
                                start=(r == 0), stop=False, perf_mode=DR)
                    lt, wt = rounds[-1]
                    for n in ACC_ORDER:
                        nc.tensor.matmul(
                            accs[n][:], lhsT=lt,
                            rhs=wt[:, :, n * NQ:(n + 1) * NQ],
                            start=False, stop=True, perf_mode=DR)
                    # PSUM -> SBUF copies, staggered order matching the
                    # final round so the next chunk never waits
                    zn = wk_.tile([P, nfeat], BF, tag="work")
                    for n in ACC_ORDER:
                        if n < nln:
                            nc.scalar.copy(out=zn[:, n * NQ:(n + 1) * NQ],
                                           in_=accs[n][:])
                        elif dst_V is not None:
                            nc.scalar.activation(
                                out=dst_V[:, t, (n - nln) * NQ:
                                          (n - nln + 1) * NQ],
                                in_=accs[n][:], func=AF.Copy,
                                scale=1.0 / WSC)
                    # transposes of chunk t-3; their PSUM-drain copies ride
                    # on DVE, which is idle during this chunk's matmuls
                    if t >= 2:
                        drain(1)
                    ln_chain(zn, nln, stp, w_t, b_t, stage, eps=eps_s_t)
                    # affine on Pool + rope on DVE, split into head groups.
                    # The last two chunks split finer: their transposes are
                    # close to the stage tail, so a shorter serial chain
                    # (affine part -> rope part pipelined across engines)
                    # directly shortens the stage-exit stall.
                    nsplit = 4 if t >= nchunks - 2 else 2
                    ohs = out_heads // nsplit
                    zbf = bfp.tile([P, out_heads, HD], BF, tag="zbf")
                    zv = zn.rearrange("p (h f two) -> p h f two",
                                      h=out_heads, two=2)
                    zb = zbf.rearrange("p h (f two) -> p h f two", two=2)
                    ct_t, st_t = cs_tiles[t // 8]
                    shp = (P, ohs, HD // 2)
                    cb = ct_t[:, t % 8, :][:, None, :].to_broadcast(shp)
                    sb = st_t[:, t % 8, :][:, None, :].to_broadcast(shp)
                    for part in range(nsplit):
                        f0 = part * (nfeat // nsplit)
                        f1 = (part + 1) * (nfeat // nsplit)
                        nc.gpsimd.tensor_mul(out=zn[:, f0:f1],
                                             in0=zn[:, f0:f1],
                                             in1=w_t[:, f0:f1])
                        nc.gpsimd.tensor_add(out=zn[:, f0:f1],
                                             in0=zn[:, f0:f1],
                                             in1=b_t[:, f0:f1])
                        h0, h1 = part * ohs, (part + 1) * ohs
                        re = zv[:, h0:h1, :, 0]
                        im = zv[:, h0:h1, :, 1]
                        rebf = zb[:, h0:h1, :, 0]
                        imbf = zb[:, h0:h1, :, 1]
                        t1 = rtp.tile([P, ohs, HD // 2], BF, tag="r1")
                        t2 = rtp.tile([P, ohs, HD // 2], BF, tag="r2")
                        nc.vector.tensor_mul(out=t1, in0=re, in1=cb)
                        nc.vector.tensor_mul(out=t2, in0=im, in1=sb)
                        nc.vector.tensor_sub(out=rebf, in0=t1, in1=t2)
                        nc.vector.tensor_mul(out=t1, in0=re, in1=sb)
                        nc.vector.tensor_mul(out=t2, in0=im, in1=cb)
                        nc.vector.tensor_add(out=imbf, in0=t1, in1=t2)
                    pending.append((zbf, t * P))
                while len(pending) > keep_last:
                    drain(1)
                for pool in (psP, rtp, stp, wk_, csp):
                    pool.release()
                if keep_last == 0:
                    if bfp_ext is None:
                        bfp.release()
                    return [], None
                # bfp (bottom of this stage's transient stack) stays alive
                # so the caller can emit the kept chunks' transposes later
                return pending, bfp

            # =========================================================
            # Stage Q: local-half Q projection
            # =========================================================
            lnq = tc.alloc_tile_pool(name="lnq", bufs=1)
            qw_t = bcast_vec(lnq, qw_d, D)
            qb_t = bcast_vec(lnq, qb_d, D)
            xsQ = tc.alloc_tile_pool(name="xsQ", bufs=2)
            xq0h = xsQ.tile([P, DC, P], F8, tag="xth", name="xthQ_0")
            nc.sync.dma_start(out=xq0h, in_=xqh_d[0])
            xq0l = xsQ.tile([P, DC, P], F8, tag="xtl", name="xtlQ_0")
            nc.sync.dma_start(out=xq0l, in_=xql_d[0])
            xq1h = xsQ.tile([P, DC, P], F8, tag="xth", name="xthQ_1")
            nc.sync.dma_start(out=xq1h, in_=xqh_d[1])
            xq1l = xsQ.tile([P, DC, P], F8, tag="xtl", name="xtlQ_1")
            nc.sync.dma_start(out=xq1l, in_=xql_d[1])
            wQ = tc.alloc_tile_pool(name="wQ", bufs=1)
            wq_sb = []
            for g in range(DC // 2):
                wth = wQ.tile([P, 2, D], F8, tag=f"wqh{g}", name=f"wqh{g}")
                nc.sync.dma_start(out=wth, in_=wqh_d[g])
                wtl = wQ.tile([P, 2, D], F8, tag=f"wql{g}", name=f"wql{g}")
                nc.sync.dma_start(out=wtl, in_=wql_d[g])
                wq_sb.append((wth, wtl))

            # prefetch first half of wkv into untouched right-side SBUF
            wKVa = tc.alloc_tile_pool(name="wKVa", bufs=1, side="right")
            wkv_sb = [None] * (DC // 2)

            def hook_ytr(psP):
                for hg in range(2):
                    tp = psP.tile([P, 4, P], BF, tag="tr", bufs=2,
                                  name=f"ytr{hg}")
                    for j in range(4):
                        nc.tensor.transpose(
                            tp[:, j, :], ykbf[:, hg * 4 + j, :], ident)
                    nc.scalar.copy(
                        out=YKT[:, hg * 4:(hg + 1) * 4, :], in_=tp)

            def hook_wkva(psP):
                for g in range(4):
                    wth = wKVa.tile([P, 2, 2 * KVD], F8, tag=f"wkvh{g}",
                                    name=f"wkvh{g}")
                    nc.sync.dma_start(out=wth, in_=wkvh_d[g])
                    wtl = wKVa.tile([P, 2, 2 * KVD], F8, tag=f"wkvl{g}",
                                    name=f"wkvl{g}")
                    nc.sync.dma_start(out=wtl, in_=wkvl_d[g])
                    wkv_sb[g] = (wth, wtl)

            q_pending, _ = proj_stage(
                TCL, (xqh_d, xql_d), wq_sb, H, qw_t, qb_t,
                cosq_d, sinq_d, QT, None, "Q",
                xsQ, {0: (xq0h, xq0l), 1: (xq1h, xq1l)},
                {1: hook_ytr, 3: hook_wkva}, keep_last=2, bfp_ext=bfQp)
            wQ.release()
            xsQ.release()
            lnq.release()

            # =========================================================
            # Stage KV: full-seq K (LN+rope) and V projections, one x pass
            # =========================================================
            ktvp = ctx.enter_context(tc.tile_pool(name="ktvpool", bufs=1))
            KT = ktvp.tile([P, KVH, S], BF)
            Vsb = ktvp.tile([P, TC, KVD], BF)
            lnk = tc.alloc_tile_pool(name="lnk", bufs=1)
            kw_t = bcast_vec(lnk, kw_d, KVD)
            kb_t = bcast_vec(lnk, kb_d, KVD)
            # x tiles ahead of the wkv-second-half DMAs in the SP queue
            xsK = tc.alloc_tile_pool(name="xsK", bufs=2)
            xk0h = xsK.tile([P, DC, P], F8, tag="xth", name="xthK_0")
            nc.sync.dma_start(out=xk0h, in_=xh_d[0])
            xk0l = xsK.tile([P, DC, P], F8, tag="xtl", name="xtlK_0")
            nc.sync.dma_start(out=xk0l, in_=xl_d[0])
            xk1h = xsK.tile([P, DC, P], F8, tag="xth", name="xthK_1")
            nc.sync.dma_start(out=xk1h, in_=xh_d[1])
            xk1l = xsK.tile([P, DC, P], F8, tag="xtl", name="xtlK_1")
            nc.sync.dma_start(out=xk1l, in_=xl_d[1])
            # second wkv half into fresh right-side space: its DMAs have no
            # space-dependency on the Q stage and start immediately
            wKVb = tc.alloc_tile_pool(name="wKVb", bufs=1, side="right")
            for g in range(4, DC // 2):
                wth = wKVb.tile([P, 2, 2 * KVD], F8, tag=f"wkvh{g}",
                                name=f"wkvh{g}")
                nc.sync.dma_start(out=wth, in_=wkvh_d[g])
                wtl = wKVb.tile([P, 2, 2 * KVD], F8, tag=f"wkvl{g}",
                                name=f"wkvl{g}")
                nc.sync.dma_start(out=wtl, in_=wkvl_d[g])
                wkv_sb[g] = (wth, wtl)
            # Q's last two chunks' transposes ride in the KV stage (chunks
            # 1-2) so KV's first matmuls never sit behind Q's rope tail on
            # the in-order PE queue
            def hook_qtr(i):
                def run(psP):
                    zbf, tok0 = q_pending[i]
                    for hg in range(H // 4):
                        tp = psP.tile([P, 4, P], BF, tag="tr", bufs=2,
                                      name=f"qtr{i}_{hg}")
                        for j in range(4):
                            nc.tensor.transpose(
                                tp[:, j, :], zbf[:, hg * 4 + j, :], ident)
                        nc.scalar.copy(
                            out=QT[:, hg * 4:hg * 4 + 4, tok0:tok0 + P],
                            in_=tp)
                return run

            kv_pending, bfK = proj_stage(TC, (xh_d, xl_d), wkv_sb, KVH,
                                         kw_t, kb_t,
                                         cosk_d, sink_d, KT, Vsb, "K",
                                         xsK, {0: (xk0h, xk0l),
                                               1: (xk1h, xk1l)}, keep_last=3,
                                         hooks={1: hook_qtr(0),
                                                2: hook_qtr(1)})
            wKVb.release()
            wKVa.release()

            # =========================================================
            # Stage attention: per (head, q-chunk)
            # =========================================================
            mgp = tc.alloc_tile_pool(name="merged", bufs=1)
            merged_h = mgp.tile([P, H, S_LOC], F8)
            merged_l = mgp.tile([P, H, S_LOC], F8)
            wop = tc.alloc_tile_pool(name="wop", bufs=2)
            psA = tc.alloc_tile_pool(name="psA", bufs=1, space="PSUM")
            outp = tc.alloc_tile_pool(name="outp", bufs=2)
            # loop-transient pools sit on top so they can release before
            # the out-proj tail, funding a deep wo prefetch ring there
            ep = tc.alloc_tile_pool(name="epool", bufs=4)
            esp = tc.alloc_tile_pool(name="espool", bufs=2)
            eyp = tc.alloc_tile_pool(name="eypool", bufs=2)
            rcp = tc.alloc_tile_pool(name="rcpool", bufs=1)
            # prefetch first wo slices during attention
            wo_tiles = {}
            for oc in range(2):
                wo_th = wop.tile([P, DC, P], F8, tag="woh", name=f"woh{oc}")
                nc.sync.dma_start(out=wo_th, in_=woh_d[oc])
                wo_tl = wop.tile([P, DC, P], F8, tag="wol", name=f"wol{oc}")
                nc.sync.dma_start(out=wo_tl, in_=wol_d[oc])
                wo_tiles[oc] = (wo_th, wo_tl)

            def emit_outproj_block(oc, qc):
                """24 DoubleRow matmuls into an o-ring PSUM slot + DVE
                scaled copy + DMA out. ACT-free so it absorbs the exp
                backlog when interleaved."""
                if oc in wo_tiles:
                    wo_th, wo_tl = wo_tiles.pop(oc)
                else:
                    wo_th = wop.tile([P, DC, P], F8, tag="woh",
                                     name=f"woh{oc}_{qc}")
                    nc.sync.dma_start(out=wo_th, in_=woh_d[oc])
                    wo_tl = wop.tile([P, DC, P], F8, tag="wol",
                                     name=f"wol{oc}_{qc}")
                    nc.sync.dma_start(out=wo_tl, in_=wol_d[oc])
                q0 = qc * NQ
                out_ps = psA.tile([P, NQ], F32, tag="o", bufs=2,
                                  name=f"ops{oc}_{qc}")
                rounds = []
                for i in range(DC // 2):
                    sl = slice(2 * i, 2 * i + 2)
                    rounds.append((wo_th[:, sl, :], merged_h[:, sl,
                                                            q0:q0 + NQ]))
                    rounds.append((wo_tl[:, sl, :], merged_h[:, sl,
                                                            q0:q0 + NQ]))
                    rounds.append((wo_th[:, sl, :], merged_l[:, sl,
                                                            q0:q0 + NQ]))
                for r, (lt, rt) in enumerate(rounds):
                    nc.tensor.matmul(
                        out_ps[:], lhsT=lt, rhs=rt,
                        start=(r == 0), stop=(r == len(rounds) - 1),
                        perf_mode=DR)
                out_t = outp.tile([P, NQ], F32, tag="outt")
                nc.vector.tensor_scalar_mul(out_t, out_ps[:],
                                            1.0 / (16.0 * WSC))
                nc.sync.dma_start(
                    out=outT[oc * P:(oc + 1) * P, q0:q0 + NQ],
                    in_=out_t)

            # cross scores for iteration idx+1 are issued at g5 of idx so
            # ACT's Ey exp never waits on a cold sy matmul
            def emit_sy(idx):
                qc, h = divmod(idx, H)
                kv = h // NREP
                sy = psA.tile([P, NQ], F32, tag="cr", bufs=1,
                              name=f"sy{idx}")
                nc.tensor.matmul(sy[:], lhsT=YKT[:, kv, :],
                                 rhs=QT[:, h, qc * NQ:qc * NQ + NQ],
                                 start=True, stop=True,
                                 skip_group_check=True)
                Ey = eyp.tile([P, NQ], BF, tag="Ey", name=f"Ey{idx}")
                nc.scalar.activation(out=Ey, in_=sy[:], func=AF.Exp,
                                     scale=SCALE, bias=ymb_t)
                return Ey

            def emit_tail(st):
                """Denominator tail + merge of the previous iteration,
                emitted after the next iteration's first score group so the
                exp backlog never blocks the PE at iteration boundaries."""
                es8_, d_, o_, t1_, h_, q0_ = st
                nc.tensor.matmul(d_[:], lhsT=ones_t, rhs=es8_[:, 2, :],
                                 start=False, stop=False,
                                 skip_group_check=True)
                nc.tensor.matmul(d_[:], lhsT=ones_t, rhs=es8_[:, 3, :],
                                 start=False, stop=False,
                                 skip_group_check=True)
                nc.tensor.matmul(d_[:], lhsT=ones_t, rhs=es8_[:, 4, :],
                                 start=False, stop=True,
                                 skip_group_check=True)
                rec = rcp.tile([P, NQ], F32, tag="rec")
                nc.vector.reciprocal(out=rec, in_=d_[:])
                t0 = rcp.tile([P, NQ], F32, tag="t0")
                nc.vector.tensor_mul(out=t0, in0=o_[:], in1=rec)
                # merged (x16 via the 1/16 ones) as fp8 hi + lo residual;
                # t0 doubles as the f32 sum (in-place add)
                nc.gpsimd.tensor_add(out=t0, in0=t0, in1=t1_)
                nc.gpsimd.tensor_copy(out=merged_h[:, h_, q0_:q0_ + NQ],
                                      in_=t0)
                nc.gpsimd.tensor_sub(out=merged_l[:, h_, q0_:q0_ + NQ],
                                     in0=t0,
                                     in1=merged_h[:, h_, q0_:q0_ + NQ])

            NIT = QCN * H
            Ey = emit_sy(0)
            deferred = None
            for idx in range(NIT):
                qc, h = divmod(idx, H)
                kv = h // NREP
                q0 = qc * NQ
                qt = QT[:, h, q0:q0 + NQ]
                o_ps = psA.tile([P, NQ], F32, tag="o", bufs=2)
                d_ps = None
                # compact 5-slot sum ring: g->slot [0,1,1,2,2,3,3,4]; the in-place
                # tree (g1,3,5) folds pairs into slots 0-2, freeing 3 slots
                es8 = esp.tile([P, 5, NQ], BF, tag="es8")

                # AV rides one group behind the scores: while ACT exps
                # group g, the PE runs group g+1's score matmuls instead of
                # head-of-line-blocking on AV(g); AV(g) lands after them.
                def emit_av(E_, g_):
                    for j in range(2):
                        kc = 2 * g_ + j
                        nc.tensor.matmul(
                            o_ps[:],
                            lhsT=Vsb[:, kc, kv * HD:(kv + 1) * HD],
                            rhs=E_[:, j, :],
                            start=(kc == 0), stop=(kc == TC - 1),
                            skip_group_check=True)

                prev_E = None
                for g in range(8):
                    if idx == 0 and g >= 5 and kv_pending:
                        # chunks 14/15's K transposes, deferred past their
                        # rope chains; first consumer is g7's score mms
                        zbf15, ktok = kv_pending.pop(0)
                        for hg in range(2):
                            tp = psA.tile([P, 4, P], BF, tag="s", bufs=2,
                                          name=f"ktr{ktok}_{hg}")
                            for j in range(4):
                                nc.tensor.transpose(
                                    tp[:, j, :], zbf15[:, hg * 4 + j, :],
                                    ident)
                            nc.scalar.copy(
                                out=KT[:, hg * 4:(hg + 1) * 4,
                                       ktok:ktok + P],
                                in_=tp)
                    s_ps = psA.tile([P, 2, NQ], F32, tag="s", bufs=2)
                    for j in range(2):
                        kc = 2 * g + j
                        nc.tensor.matmul(
                            s_ps[:, j, :],
                            lhsT=KT[:, kv, kc * P:(kc + 1) * P],
                            rhs=qt, start=True, stop=True,
                            skip_group_check=True)
                    if prev_E is not None:
                        emit_av(prev_E, g - 1)
                    E = ep.tile([P, 2, NQ], BF, tag="E")
                    nc.scalar.activation(out=E, in_=s_ps[:],
                                         func=AF.Exp, scale=SCALE)
                    nc.vector.tensor_add(
                        out=es8[:, (g + 1) // 2, :],
                        in0=E[:, 0, :], in1=E[:, 1, :])
                    prev_E = E
                    if g == 0 and deferred is not None:
                        emit_tail(deferred)
                        deferred = None
                    if g in (1, 3, 5):
                        # lvl1 tree add on Pool (in-place into es8[0:3])
                        i = g // 2
                        nc.gpsimd.tensor_add(out=es8[:, i, :],
                                             in0=es8[:, i, :],
                                             in1=es8[:, i + 1, :])
                    if g == 1:
                        dy = psA.tile([P, NQ], F32, tag="cr", bufs=1,
                                      name=f"dy{idx}")
                        nc.tensor.matmul(dy[:], lhsT=ones_t, rhs=Ey,
                                         start=True, stop=True,
                                         skip_group_check=True)
                        rec_y = rcp.tile([P, NQ], F32, tag="recy")
                        nc.vector.reciprocal(out=rec_y, in_=dy[:])
                    elif g == 3:
                        d_ps = psA.tile([P, NQ], F32, tag="d", bufs=1,
                                        name=f"d{idx}")
                        nc.tensor.matmul(
                            d_ps[:], lhsT=ones_t, rhs=es8[:, 0, :],
                            start=True, stop=False,
                            skip_group_check=True)
                        oy = psA.tile([P, NQ], F32, tag="cr", bufs=1,
                                      name=f"oy{idx}")
                        nc.tensor.matmul(oy[:], lhsT=YV[:, kv, :],
                                         rhs=Ey, start=True, stop=True,
                                         skip_group_check=True)
                        t1 = rcp.tile([P, NQ], F32, tag="t1", bufs=2)
                        nc.vector.scalar_tensor_tensor(
                            out=t1, in0=oy[:],
                            scalar=gates_t[:, h:h + 1],
                            in1=rec_y, op0=ALU.mult, op1=ALU.mult)
                    elif g == 5:
                        nc.tensor.matmul(
                            d_ps[:], lhsT=ones_t, rhs=es8[:, 1, :],
                            start=False, stop=False,
                            skip_group_check=True)
                        if idx + 1 < NIT:
                            next_Ey = emit_sy(idx + 1)

                emit_av(prev_E, 7)
                deferred = (es8, d_ps, o_ps, t1, h, q0)
                Ey = next_Ey
                # during the second q-chunk pass, interleave the first
                # q-chunk's output projection (ACT-free PE work)
                if qc == 1:
                    emit_outproj_block(h, 0)

            if deferred is not None:
                emit_tail(deferred)
                deferred = None
            # loop transients are dead now; a deep wo ring keeps the tail's
            # DMAs ahead of its back-to-back PE blocks
            rcp.release()
            eyp.release()
            esp.release()
            ep.release()
            wot = tc.alloc_tile_pool(name="wot", bufs=6)
            for oc in range(DC):
                if oc not in wo_tiles:
                    wo_th = wot.tile([P, DC, P], F8, tag="woh",
                                     name=f"twoh{oc}")
                    nc.sync.dma_start(out=wo_th, in_=woh_d[oc])
                    wo_tl = wot.tile([P, DC, P], F8, tag="wol",
                                     name=f"twol{oc}")
                    nc.sync.dma_start(out=wo_tl, in_=wol_d[oc])
                    wo_tiles[oc] = (wo_th, wo_tl)
                emit_outproj_block(oc, 1)
            wot.release()
            psA.release()
            outp.release()
            wop.release()
            mgp.release()
            bfK.release()
            xsK.release()
            lnk.release()

    _split_dma_waits(nc)
    return nc


def _prep_inputs(x, y, freqs_cos, freqs_sin, y_mask, wq, wk, wv, wk_y, wv_y,
                 wo, q_w, q_b, k_w, k_b, ky_w, ky_b, gate):
    f32 = np.float32

    def hilo(v):
        hi = v.astype(F8NP)
        lo = (v - hi.astype(f32)).astype(F8NP)
        return hi, lo

    def chunk_x(xb):
        # [S, D] -> [tc, p, dc, s]: out[t, p, dc, s] = xb[t*128+s, dc*128+p]
        # as an fp8 (hi, lo) pair
        t = xb.shape[0] // P
        ch = np.ascontiguousarray(
            xb.reshape(t, P, DC, P).transpose(0, 3, 2, 1))
        return hilo(ch)

    def pair_w(wm):
        # [D, F] -> fp8 hi/lo pairs [DC/2, p, 2, F] (WSC pre-scale)
        fdim = wm.shape[1]
        ws = np.ascontiguousarray(
            (np.asarray(wm, f32) * WSC).reshape(DC // 2, 2, P, fdim)
            .transpose(0, 2, 1, 3))
        return hilo(ws)

    def chunk_cs(tab):
        # [S', 64] -> [p, t, f]
        t = tab.shape[0] // P
        return np.ascontiguousarray(
            np.asarray(tab, f32).reshape(t, P, HD // 2).transpose(1, 0, 2)
            .astype(BF16))

    wo_s = np.ascontiguousarray(
        (np.asarray(wo, f32) * WSC).reshape(DC, P, DC, P)
        .transpose(2, 1, 0, 3))
    woh8 = wo_s.astype(F8NP)
    wol8 = (wo_s - woh8.astype(f32)).astype(F8NP)
    wqh, wql = pair_w(np.asarray(wq, f32))
    wkvh, wkvl = pair_w(np.concatenate(
        [np.asarray(wk, f32), np.asarray(wv, f32)], axis=1))
    shared = {
        "wqh": wqh,
        "wql": wql,
        "wkvh": wkvh,
        "wkvl": wkvl,
        "wkvy": np.ascontiguousarray(np.concatenate(
            [np.asarray(wk_y, f32), np.asarray(wv_y, f32)],
            axis=1).astype(BF16).reshape(YDC, P, 2 * KVD)),
        "woh": woh8,
        "wol": wol8,
        "qw": np.ascontiguousarray(np.asarray(q_w, f32).astype(BF16)),
        "qb": np.ascontiguousarray(np.asarray(q_b, f32).astype(BF16)),
        "kw": np.ascontiguousarray(np.asarray(k_w, f32).astype(BF16)),
        "kb": np.ascontiguousarray(np.asarray(k_b, f32).astype(BF16)),
        "kyw": np.ascontiguousarray(np.asarray(ky_w, f32).astype(BF16)),
        "kyb": np.ascontiguousarray(np.asarray(ky_b, f32).astype(BF16)),
        "cosk": chunk_cs(freqs_cos),
        "sink": chunk_cs(freqs_sin),
        "gates": np.ascontiguousarray(np.tanh(np.asarray(gate, f32))),
    }
    per_core = []
    for c in range(8):
        b, hf = c // 2, c % 2
        sl = slice(hf * S_LOC, (hf + 1) * S_LOC)
        xb = np.asarray(x[b], f32)
        m = dict(shared)
        m["xh"], m["xl"] = chunk_x(xb)
        m["xqh"], m["xql"] = chunk_x(xb[sl])
        m["y"] = np.ascontiguousarray(
            np.asarray(y[b], f32).T.astype(BF16).reshape(YDC, P, YL)
            .transpose(1, 0, 2))
        m["cosq"] = chunk_cs(np.asarray(freqs_cos, f32)[sl])
        m["sinq"] = chunk_cs(np.asarray(freqs_sin, f32)[sl])
        m["ymb"] = np.where(np.asarray(y_mask[b]), 0.0, -1e9).astype(f32)
        per_core.append(m)
    return per_core


def kernel(**inputs):
    if "nc" not in _CACHED:
        _CACHED["nc"] = build_program()
    nc = _CACHED["nc"]
    in_maps = _prep_inputs(
        inputs["x"], inputs["y"], inputs["freqs_cos"], inputs["freqs_sin"],
        inputs["y_mask"], inputs["wq"], inputs["wk"], inputs["wv"],
        inputs["wk_y"], inputs["wv_y"], inputs["wo"], inputs["q_w"],
        inputs["q_b"], inputs["k_w"], inputs["k_b"], inputs["ky_w"],
        inputs["ky_b"], inputs["gate"])
    res = run_bass_kernel_spmd(nc, in_maps, core_ids=list(range(8)))
    global LAST_EXEC_NS
    LAST_EXEC_NS = res.exec_time_ns
    out = np.zeros((B, S, D), np.float32)
    for c in range(8):
        b, hf = c // 2, c % 2
        out[b, hf * S_LOC:(hf + 1) * S_LOC, :] = res.results[c]["outT"].T
    return out


if __name__ == "__main__":
    nc = build_program()
    print("program built OK")

